# revision 1
# baseline (speedup 1.0000x reference)
# GGNN encoder kernel for Trainium2 (Bass/Tile), data-parallel over the
# batch dimension: 8 graphs -> 8 NeuronCores, one graph per core.
#
# Per-core computation (one graph):
#   type_e  = type_table[node_types]                       # [N, TD]
#   tok_e   = word_emb[node_token_ids]                     # [T, D]   (SWDGE dma_gather)
#   text_e  = segment_mean(tok_e, token_seg_ids)           # [N, D]   (PE matmul w/ pooling matrix)
#   h       = concat(type_e, text_e) @ fusion_w + b        # [N, D]
#   4 x GGNN layer:
#     m    = h @ Wl                                        # [N, D]
#     agg  = A @ m          (A dense adjacency, built host-side from edge list)
#     GRU(h, agg)
#   out     = mask * h
#
# Layout strategy: h, agg, gates are kept feature-major ("T" layout,
# [feat partitions, node free-dim]) so that the feature-contracting GRU
# matmuls can run directly; m is node-major for the node-contracting
# scatter matmul. Matmuls run as float32r (full fp32 storage, single-pass
# PE mode) for 4x throughput over plain fp32.

import functools

import numpy as np

import concourse.bass as bass
import concourse.mybir as mybir
import concourse.tile as tile
from concourse import bacc, bass_utils
from concourse.masks import make_identity

# Problem shapes (hardcoded: kernel must be self-contained).
B, N, T, D, TD, L = 8, 512, 2048, 768, 128, 4
V, TYPES = 30522, 64
MAX_NODE_LEN = 512
K3 = 3 * D            # 2304 stacked GRU gate rows
F = TD + D            # 896 fused embedding dim
P = 128               # partitions
NCH = N // P          # 4 node chunks
TCH = T // P          # 16 token chunks
DCH = D // P          # 6 feature chunks
FCH = F // P          # 7 fused-dim chunks
GCH = 3 * DCH         # 18 gate row chunks
BLK = N // TCH        # 32 nodes per token chunk (block-pooling case)
NF = 512              # free-dim tile (nodes)
GS = 4                # token gather splits
GT = T // GS          # tokens per gather split (512)
GC = GT // P          # 128-chunks per gather split (4)

f32 = mybir.dt.float32
f32r = mybir.dt.float32r
i32 = mybir.dt.int32
i16 = mybir.dt.int16

Sigmoid = mybir.ActivationFunctionType.Sigmoid
Tanh = mybir.ActivationFunctionType.Tanh
Ident = mybir.ActivationFunctionType.Identity


def build_nc(pool_wide: bool) -> bass.Bass:
    nc = bacc.Bacc(num_swdge_queues=2, dynamic_dma_scratch_size=32768)

    # All host-side tensors are pre-laid-out partition-major so every DMA is
    # contiguous per partition.
    tok_idx = nc.dram_tensor("tok_idx", [P, GS * (GT // 16)], i16,
                             kind="ExternalInput")  # [128, 4*32] wrapped idxs
    typ_oh = nc.dram_tensor("typ_oh", [TYPES, N], f32r, kind="ExternalInput")
    word_emb = nc.dram_tensor("word_emb", [V, D], f32r, kind="ExternalInput")
    type_table = nc.dram_tensor("type_table", [TYPES, TD], f32r, kind="ExternalInput")
    pool_w = N if pool_wide else BLK
    poolm = nc.dram_tensor("poolm", [P, TCH, pool_w], f32r, kind="ExternalInput")
    at_w = nc.dram_tensor("at_w", [P, NCH, N], f32r, kind="ExternalInput")
    fusion_w = nc.dram_tensor("fusion_w", [F, D], f32r, kind="ExternalInput")
    fusion_b = nc.dram_tensor("fusion_b", [P, DCH], f32, kind="ExternalInput")
    wl = nc.dram_tensor("wl", [L, DCH, P, D], f32r, kind="ExternalInput")
    wih = nc.dram_tensor("wih", [P, DCH, K3], f32r, kind="ExternalInput")
    whh_st = nc.dram_tensor("whh_st", [GCH, P, DCH, P], f32r, kind="ExternalInput")
    bsum = nc.dram_tensor("bsum", [P, GCH], f32, kind="ExternalInput")
    bihn = nc.dram_tensor("bihn", [P, DCH], f32, kind="ExternalInput")
    bhhn = nc.dram_tensor("bhhn", [P, DCH], f32, kind="ExternalInput")
    maskc = nc.dram_tensor("maskc", [P, NCH], f32, kind="ExternalInput")
    out = nc.dram_tensor("out", [N, D], f32, kind="ExternalOutput")

    with tile.TileContext(nc) as tc:
        with (
            tc.tile_pool(name="consts", bufs=1) as consts,
            tc.tile_pool(name="wbig", bufs=1) as wbig,
            tc.tile_pool(name="t768", bufs=7) as t768,
            tc.tile_pool(name="c512", bufs=7) as c512,
            tc.tile_pool(name="hpool", bufs=12) as hpool,
            tc.tile_pool(name="gpool", bufs=5) as gpool,
            tc.tile_pool(name="wst", bufs=3) as wst,
            tc.tile_pool(name="wlc", bufs=7) as wlc,
            tc.tile_pool(name="tokg", bufs=2) as tokg,
            tc.tile_pool(name="psA", bufs=7, space="PSUM") as psA,
        ):
            # ---- token gather first: it gates the whole front of the kernel
            tok_idx_sb = consts.tile([P, T // 16], i16)
            nc.sync.dma_start(out=tok_idx_sb[:], in_=tok_idx[:])
            pool_sb = consts.tile([P, TCH, pool_w], f32r)
            nc.sync.dma_start(out=pool_sb[:], in_=poolm[:])

            # type embeddings via one-hot matmul: two tiny DMAs + one PE op,
            # nothing queues behind the big token gathers
            tt_sb = consts.tile([TYPES, TD], f32r)
            nc.sync.dma_start(out=tt_sb[:], in_=type_table[:])
            oh_sb = consts.tile([TYPES, N], f32r)
            nc.sync.dma_start(out=oh_sb[:], in_=typ_oh[:])

            gath = []
            gath_insts = []
            for s in range(GS):
                tg = tokg.tile([P, GC, D], f32r, tag="tokg", name=f"tokg{s}")
                gi_ = nc.gpsimd.dma_gather(
                    tg[:],
                    word_emb[:],
                    tok_idx_sb[:, s * (GT // 16) : (s + 1) * (GT // 16)],
                    GT,
                    GT,
                    D,
                    queue_num=s % 2,
                )
                gath.append(tg)
                gath_insts.append(gi_)

            def after_gathers(dma_inst):
                return dma_inst

            # ---- remaining constants / small inputs ----
            identity = consts.tile([P, P], f32)
            make_identity(nc, identity[:])
            bsum_sb = consts.tile([P, GCH], f32)
            nc.sync.dma_start(out=bsum_sb[:], in_=bsum[:])
            bihn_sb = consts.tile([P, DCH], f32)
            nc.sync.dma_start(out=bihn_sb[:], in_=bihn[:])
            bhhn_sb = consts.tile([P, DCH], f32)
            nc.sync.dma_start(out=bhhn_sb[:], in_=bhhn[:])
            fb_sb = consts.tile([P, DCH], f32)
            nc.sync.dma_start(out=fb_sb[:], in_=fusion_b[:])
            mask_sb = consts.tile([P, NCH], f32)
            nc.sync.dma_start(out=mask_sb[:], in_=maskc[:])

            # ---- fused embedding (feature-major [f, n]) ----
            fusedT = [
                c512.tile([P, NF], f32r, tag="c512", name=f"fusedT{k}")
                for k in range(FCH)
            ]

            # weight loads, emitted in the order the compute will need them
            # (the DMA engines drain roughly in emission order)
            fw = []
            for k in range(FCH):
                fwk = t768.tile([P, D], f32r, tag="t768", name=f"fw{k}")
                after_gathers(nc.scalar.dma_start(
                    out=fwk[:], in_=fusion_w[k * P : (k + 1) * P, :]
                ))
                fw.append(fwk)
            wlk = []
            for k in range(DCH):
                wk = wlc.tile([P, D], f32r, tag="wlc", name=f"wl0_{k}")
                after_gathers(nc.scalar.dma_start(out=wk[:], in_=wl[0, k]))
                wlk.append(wk)
            at_sb = wbig.tile([P, NCH, N], f32r)
            after_gathers(nc.scalar.dma_start(out=at_sb[:], in_=at_w[:]))
            wih_sb = wbig.tile([P, DCH, K3], f32r)

            # type_eT = type_table.T @ onehot  (one matmul, K=64)
            ptyp = psA.tile([P, NF], f32, tag="psA")
            nc.tensor.matmul(
                out=ptyp[:], lhsT=tt_sb[:], rhs=oh_sb[:], start=True, stop=True
            )
            nc.vector.tensor_copy(out=fusedT[0][:], in_=ptyp[:])

            # token pooling: PE matmul pools 128 tokens -> 32 nodes and
            # transposes to feature-major in one pass
            for s in range(GS):
                tg = gath[s]
                for c2 in range(GC):
                    c = s * GC + c2
                    if pool_wide:
                        for f in range(DCH):
                            pc = psA.tile([P, NF], f32, tag="psA")
                            nc.tensor.matmul(
                                out=pc[:],
                                lhsT=tg[:, c2, f * P : (f + 1) * P],
                                rhs=pool_sb[:, c, :],
                                start=True,
                                stop=True,
                            )
                            if c == 0:
                                nc.vector.tensor_copy(out=fusedT[1 + f][:], in_=pc[:])
                            else:
                                nc.vector.tensor_add(
                                    out=fusedT[1 + f][:],
                                    in0=fusedT[1 + f][:],
                                    in1=pc[:],
                                )
                    else:
                        pc = psA.tile([P, DCH * BLK], f32, tag="psA")
                        for f in range(DCH):
                            nc.tensor.matmul(
                                out=pc[:, f * BLK : (f + 1) * BLK],
                                lhsT=tg[:, c2, f * P : (f + 1) * P],
                                rhs=pool_sb[:, c, :],
                                start=True,
                                stop=True,
                            )
                        for f in range(DCH):
                            nc.vector.tensor_copy(
                                out=fusedT[1 + f][:, c * BLK : (c + 1) * BLK],
                                in_=pc[:, f * BLK : (f + 1) * BLK],
                            )

            # ---- fusion matmul: hT[j] = (fusion_w.T @ fusedT)[j] + b ----
            hT = []
            for j in range(DCH):
                pf = psA.tile([P, NF], f32, tag="psA")
                for k in range(FCH):
                    nc.tensor.matmul(
                        out=pf[:],
                        lhsT=fw[k][:, j * P : (j + 1) * P],
                        rhs=fusedT[k][:],
                        start=(k == 0),
                        stop=(k == FCH - 1),
                    )
                hj = hpool.tile([P, NF], f32r, tag="hpool")
                nc.scalar.activation(
                    out=hj[:], in_=pf[:], func=Ident, bias=fb_sb[:, j : j + 1]
                )
                hT.append(hj)
                after_gathers(nc.scalar.dma_start(out=wih_sb[:, j, :], in_=wih[:, j, :]))

            # ---- GGNN layers ----
            for l in range(L):
                # m = h @ Wl   (node-major out, [node 128, 768] per chunk)
                if l > 0:
                    wlk = []
                    for k in range(DCH):
                        wk = wlc.tile([P, D], f32r, tag="wlc", name=f"wl{l}_{k}")
                        nc.scalar.dma_start(out=wk[:], in_=wl[l, k])
                        wlk.append(wk)
                m_sb = []
                for i in range(NCH):
                    pma = psA.tile([P, NF], f32, tag="psA")
                    pmb = psA.tile([P, D - NF], f32, tag="psA")
                    for k in range(DCH):
                        nc.tensor.matmul(
                            out=pma[:],
                            lhsT=hT[k][:, i * P : (i + 1) * P],
                            rhs=wlk[k][:, :NF],
                            start=(k == 0),
                            stop=(k == DCH - 1),
                        )
                        nc.tensor.matmul(
                            out=pmb[:],
                            lhsT=hT[k][:, i * P : (i + 1) * P],
                            rhs=wlk[k][:, NF:D],
                            start=(k == 0),
                            stop=(k == DCH - 1),
                        )
                    mi = t768.tile([P, D], f32r, tag="t768", name=f"m{l}_{i}")
                    nc.vector.tensor_copy(out=mi[:, :NF], in_=pma[:])
                    nc.vector.tensor_copy(out=mi[:, NF:D], in_=pmb[:])
                    m_sb.append(mi)

                # aggT = m.T @ A.T  (feature-major [feat 128, nodes 512])
                aggT = []
                for j in range(DCH):
                    pa = psA.tile([P, NF], f32, tag="psA")
                    for k in range(NCH):
                        nc.tensor.matmul(
                            out=pa[:],
                            lhsT=m_sb[k][:, j * P : (j + 1) * P],
                            rhs=at_sb[:, k, :],
                            start=(k == 0),
                            stop=(k == NCH - 1),
                        )
                    aj = c512.tile([P, NF], f32r, tag="c512", name=f"agg{l}_{j}")
                    nc.vector.tensor_copy(out=aj[:], in_=pa[:])
                    aggT.append(aj)

                # GRU gates, 128 gate rows at a time
                hnew = []
                for i in range(DCH):
                    # streamed Whh chunks for the three gates at row-chunk i
                    wch = []
                    for g in range(3):
                        w = wst.tile([P, DCH, P], f32r, tag="wst",
                                     name=f"wch{l}_{i}_{g}")
                        wdma = nc.sync.dma_start(out=w[:], in_=whh_st[g * DCH + i])
                        if l == 0 and i == 0:
                            after_gathers(wdma)
                        wch.append(w)

                    # r and z: psum accumulates gi + gh, ACT adds bias+sigmoid
                    rz = []
                    for g in range(2):
                        pg = psA.tile([P, NF], f32, tag="psA")
                        col = g * D + i * P
                        # gh first: it only needs h + the small whh stream,
                        # so it runs while wih/aggT are still in flight
                        for k in range(DCH):
                            nc.tensor.matmul(
                                out=pg[:],
                                lhsT=wch[g][:, k, :],
                                rhs=hT[k][:],
                                start=(k == 0),
                                stop=False,
                            )
                        for k in range(DCH):
                            nc.tensor.matmul(
                                out=pg[:],
                                lhsT=wih_sb[:, k, col : col + P],
                                rhs=aggT[k][:],
                                start=False,
                                stop=(k == DCH - 1),
                            )
                        gs = gpool.tile([P, NF], f32, tag="gpool",
                                        name=f"g{l}_{i}_{g}")
                        nc.scalar.activation(
                            out=gs[:],
                            in_=pg[:],
                            func=Sigmoid,
                            bias=bsum_sb[:, g * DCH + i : g * DCH + i + 1],
                        )
                        rz.append(gs)
                    r_sb, z_sb = rz

                    # n gate: keep gi and gh separate
                    col = 2 * D + i * P
                    pghn = psA.tile([P, NF], f32, tag="psA")
                    for k in range(DCH):
                        nc.tensor.matmul(
                            out=pghn[:],
                            lhsT=wch[2][:, k, :],
                            rhs=hT[k][:],
                            start=(k == 0),
                            stop=(k == DCH - 1),
                        )
                    pgin = psA.tile([P, NF], f32, tag="psA")
                    for k in range(DCH):
                        nc.tensor.matmul(
                            out=pgin[:],
                            lhsT=wih_sb[:, k, col : col + P],
                            rhs=aggT[k][:],
                            start=(k == 0),
                            stop=(k == DCH - 1),
                        )
                    hb = gpool.tile([P, NF], f32, tag="gpool")
                    nc.scalar.activation(
                        out=hb[:], in_=pghn[:], func=Ident,
                        bias=bhhn_sb[:, i : i + 1],
                    )
                    rn = gpool.tile([P, NF], f32, tag="gpool")
                    nc.vector.tensor_mul(out=rn[:], in0=r_sb[:], in1=hb[:])
                    tn = gpool.tile([P, NF], f32, tag="gpool")
                    nc.vector.tensor_add(out=tn[:], in0=pgin[:], in1=rn[:])
                    nn_ = gpool.tile([P, NF], f32, tag="gpool")
                    nc.scalar.activation(
                        out=nn_[:], in_=tn[:], func=Tanh,
                        bias=bihn_sb[:, i : i + 1],
                    )
                    # h' = n + z * (h - n)
                    s_ = gpool.tile([P, NF], f32, tag="gpool")
                    nc.vector.tensor_sub(out=s_[:], in0=hT[i][:], in1=nn_[:])
                    sz = gpool.tile([P, NF], f32, tag="gpool")
                    nc.vector.tensor_mul(out=sz[:], in0=z_sb[:], in1=s_[:])
                    hj = hpool.tile([P, NF], f32r, tag="hpool",
                                    name=f"h{l}_{i}")
                    nc.vector.tensor_add(out=hj[:], in0=nn_[:], in1=sz[:])
                    hnew.append(hj)
                hT = hnew

            # ---- transpose back to node-major, mask, write out ----
            for i in range(NCH):
                poa = psA.tile([P, NF], f32, tag="psA")
                pob = psA.tile([P, D - NF], f32, tag="psA")
                for j in range(DCH):
                    dst = poa[:, j * P : (j + 1) * P] if j < 4 else \
                        pob[:, (j - 4) * P : (j - 3) * P]
                    nc.tensor.transpose(
                        out=dst,
                        in_=hT[j][:, i * P : (i + 1) * P].bitcast(f32),
                        identity=identity[:],
                    )
                ob = t768.tile([P, D], f32, tag="t768")
                nc.vector.tensor_scalar_mul(
                    out=ob[:, :NF], in0=poa[:], scalar1=mask_sb[:, i : i + 1]
                )
                nc.vector.tensor_scalar_mul(
                    out=ob[:, NF:D], in0=pob[:], scalar1=mask_sb[:, i : i + 1]
                )
                nc.sync.dma_start(out=out[i * P : (i + 1) * P, :], in_=ob[:])

    nc.compile()
    return nc


@functools.lru_cache(maxsize=2)
def _get_nc(pool_wide: bool) -> bass.Bass:
    return build_nc(pool_wide)


def _prep_shared(inputs):
    """Weight tensors identical across graphs, pre-laid-out partition-major."""
    fusion_w = np.ascontiguousarray(np.asarray(inputs["fusion_w"], np.float32))
    fusion_b = np.ascontiguousarray(
        np.asarray(inputs["fusion_b"], np.float32).reshape(DCH, P).T
    )
    wl = np.ascontiguousarray(
        np.asarray(inputs["ggnn_w"], np.float32).reshape(L, DCH, P, D)
    )
    wih_w = np.asarray(inputs["gru_w_ih"], np.float32)   # [K3, D]
    whh_w = np.asarray(inputs["gru_w_hh"], np.float32)
    bih = np.asarray(inputs["gru_b_ih"], np.float32)
    bhh = np.asarray(inputs["gru_b_hh"], np.float32)
    # wih: [P, DCH, K3]  (partition p, feat chunk k -> gate rows)
    wihT = wih_w.T                                       # [D, K3]
    wih = np.ascontiguousarray(wihT.reshape(DCH, P, K3).transpose(1, 0, 2))
    # whh chunks: [GCH, P, DCH, P]
    whhT = whh_w.T                                       # [D, K3]
    whh_st = np.ascontiguousarray(
        np.stack(
            [
                whhT[:, j * P : (j + 1) * P].reshape(DCH, P, P).transpose(1, 0, 2)
                for j in range(GCH)
            ]
        )
    )
    bsum = np.ascontiguousarray((bih + bhh).reshape(GCH, P).T)
    bihn = np.ascontiguousarray(bih[2 * D :].reshape(DCH, P).T)
    bhhn = np.ascontiguousarray(bhh[2 * D :].reshape(DCH, P).T)
    word_emb = np.ascontiguousarray(np.asarray(inputs["word_emb"], np.float32))
    type_table = np.ascontiguousarray(np.asarray(inputs["type_table"], np.float32))
    return dict(
        word_emb=word_emb, type_table=type_table, fusion_w=fusion_w,
        fusion_b=fusion_b, wl=wl, wih=wih, whh_st=whh_st, bsum=bsum,
        bihn=bihn, bhhn=bhhn,
    )


def _graph_blockable(inputs, b):
    seg = np.asarray(inputs["token_seg_ids"][b], np.int64)
    tcol = np.arange(T) // P
    return bool(np.all((seg >= tcol * BLK) & (seg < (tcol + 1) * BLK)))


def _prep_graph(inputs, b, pool_wide):
    tok = np.asarray(inputs["node_token_ids"][b], np.int64)
    typ = np.asarray(inputs["node_types"][b], np.int32)
    seg = np.asarray(inputs["token_seg_ids"][b], np.int64)
    lens = np.asarray(inputs["node_token_lens"][b], np.float64)
    glen = int(np.asarray(inputs["graph_node_lens"][b]))
    esrc = np.asarray(inputs["edge_src"][b], np.int64)
    edst = np.asarray(inputs["edge_dst"][b], np.int64)
    ew = np.asarray(inputs["edge_weight"][b], np.float32)

    # token idxs for dma_gather: GS splits of GT idxs, each wrapped into
    # 16 partitions ([p, s] = idx[s*16+p]) and replicated to 128 partitions
    tok16 = tok.astype(np.int16)
    cols = []
    for s in range(GS):
        w16 = tok16[s * GT : (s + 1) * GT].reshape(GT // 16, 16).T  # [16, GT/16]
        cols.append(np.tile(w16, (8, 1)))                           # [128, GT/16]
    tok_idx = np.ascontiguousarray(np.concatenate(cols, axis=1))    # [128, GS*32]

    typ_oh = np.zeros((TYPES, N), np.float32)
    typ_oh[typ, np.arange(N)] = 1.0

    # dense transposed adjacency: AT[src, dst], laid out [P, NCH, N]
    at = np.zeros((N, N), np.float32)
    np.add.at(at, (esrc, edst), ew)
    at = np.ascontiguousarray(at.reshape(NCH, P, N).transpose(1, 0, 2))

    # pooling matrix (1/len weights), [P, TCH, BLK or N]
    winv = np.zeros(N, np.float64)
    nzmask = lens != 0
    winv[nzmask] = 1.0 / lens[nzmask]
    tcol = np.arange(T) // P  # token chunk of each token
    if pool_wide:
        poolm = np.zeros((TCH, P, N), np.float32)
        poolm[tcol, np.arange(T) % P, seg] = winv[seg]
    else:
        poolm = np.zeros((TCH, P, BLK), np.float32)
        poolm[tcol, np.arange(T) % P, seg - tcol * BLK] = winv[seg]
    poolm = np.ascontiguousarray(poolm.transpose(1, 0, 2))

    keep = min(glen, MAX_NODE_LEN)
    mask = np.ascontiguousarray(
        (np.arange(N) < keep).astype(np.float32).reshape(NCH, P).T
    )
    return dict(tok_idx=tok_idx, typ_oh=typ_oh, at_w=at, poolm=poolm,
                maskc=mask)


def kernel(**inputs) -> np.ndarray:
    shared = _prep_shared(inputs)
    pool_wide = not all(_graph_blockable(inputs, b) for b in range(B))
    per_graph = [_prep_graph(inputs, b, pool_wide) for b in range(B)]
    nc = _get_nc(pool_wide)
    in_maps = [{**shared, **per_graph[b]} for b in range(B)]
    res = bass_utils.run_bass_kernel_spmd(nc, in_maps, core_ids=list(range(B)))
    global _last_exec_ns
    _last_exec_ns = res.exec_time_ns
    out = np.stack([r["out"] for r in res.results]).astype(np.float32)
    return out


_last_exec_ns = None



# revision 3
# speedup vs baseline: 1.6566x; 1.6566x over previous
# GGNN encoder kernel for Trainium2 (Bass/Tile), data-parallel over the
# batch dimension: 8 graphs -> 8 NeuronCores, one graph per core.
#
# Mixed-precision design ("R2"):
#  - r/z gates and gh_n run as fp8(e4m3) DoubleRow matmuls (two 128-deep
#    K-planes per instruction at 0.5 cycles/row).  Their quantization
#    noise is squashed by the sigmoid (and by r*gh_n being small).
#  - The sensitive n-gate input gi_n = A^T (h @ Wl @ Wih_n^T) is computed
#    exactly in f32r via a host-side weight fold (Wcn = Wl @ Wih_n^T),
#    which also deletes the separate per-layer m matmul from this path.
#  - h master stays fp32; embeddings/pooling/fusion run in bf16/f32r.
#  - All fp8 operands carry power-of-two scales, folded exactly into the
#    activation-function scale arguments.
#
# Per-core computation (one graph):
#   type_e  = type_table[node_types]                       # f32r one-hot matmul
#   tok_e   = word_emb[node_token_ids]                     # bf16 SWDGE dma_gather
#   text_e  = segment_mean(tok_e, token_seg_ids)           # bf16 pooling matmul
#   h       = concat(type_e, text_e) @ fusion_w + b        # bf16 matmul
#   4 x GGNN layer:
#     m8   = h8 @ Wl8          (fp8 DR)      -> agg8 = A8-contract (fp8 DR)
#     r,z  = sigmoid(Wih8 agg8 + Whh8 h8 + b)              (fp8 DR psum)
#     gin  = A-contract(h @ Wcn)                           (f32r, exact)
#     ghn  = Whh_n8 h8                                     (fp8 DR)
#     n    = tanh(gin + r*ghn + b);  h' = n + z*(h - n)    (fp32 pointwise)
#   out     = mask * h                                     # PE transpose epilogue

import functools

import numpy as np
import ml_dtypes

import concourse.bass as bass
import concourse.mybir as mybir
import concourse.tile as tile
from concourse import bacc, bass_utils
from concourse.masks import make_identity

# Problem shapes (hardcoded: kernel must be self-contained).
B, N, T, D, TD, L = 8, 512, 2048, 768, 128, 4
V, TYPES = 30522, 64
MAX_NODE_LEN = 512
F = TD + D            # 896 fused embedding dim
P = 128               # partitions
DCH = D // P          # 6 feature chunks
KK = DCH // 2         # 3 feature chunk pairs (DoubleRow planes)
NCH = N // P          # 4 node chunks
CC = NCH // 2         # 2 node chunk pairs
FCH = F // P          # 7 fused chunks
GCH = 3 * DCH         # 18 gate row chunks
TCH = T // P          # 16 token chunks
BLK = N // TCH        # 32 nodes per token chunk (block-pooling case)
NF = 512              # free-dim tile (nodes)
GS = 4                # token gather splits
GT = T // GS          # tokens per gather split (512)
GC = GT // P          # 128-chunks per gather split (4)

# power-of-two scale exponents for the fp8 operands
AH = 4                # h -> fp8
AM = 9                # m -> fp8
AG = 4                # agg -> fp8
BWL = 5               # ggnn_w
BIH = 5               # gru_w_ih (r,z rows)
BHH = 5               # gru_w_hh
SG = BIH + AG         # gate psum scale (== BHH + AH); Wcn also pre-scaled 2^SG
assert SG == BHH + AH

f32 = mybir.dt.float32
f32r = mybir.dt.float32r
bf16 = mybir.dt.bfloat16
fp8 = mybir.dt.float8e4
i16 = mybir.dt.int16
NPF8 = ml_dtypes.float8_e4m3
NPBF = ml_dtypes.bfloat16

Sigmoid = mybir.ActivationFunctionType.Sigmoid
Tanh = mybir.ActivationFunctionType.Tanh
Ident = mybir.ActivationFunctionType.Identity
DR = mybir.MatmulPerfMode.DoubleRow


def build_nc(pool_wide: bool, has_bhhn: bool) -> bass.Bass:
    nc = bacc.Bacc(num_swdge_queues=2, dynamic_dma_scratch_size=32768)

    tok_idx = nc.dram_tensor("tok_idx", [P, T // 16], i16, kind="ExternalInput")
    typ_oh = nc.dram_tensor("typ_oh", [TYPES, N], f32r, kind="ExternalInput")
    type_table = nc.dram_tensor("type_table", [TYPES, TD], f32r,
                                kind="ExternalInput")
    if pool_wide:
        word_d = nc.dram_tensor("word_f32", [V, D], f32r, kind="ExternalInput")
        poolm = nc.dram_tensor("poolw", [P, TCH, N], f32r, kind="ExternalInput")
    else:
        word_d = nc.dram_tensor("word_bf", [V, D], bf16, kind="ExternalInput")
        poolm = nc.dram_tensor("poolm", [P, TCH, BLK], bf16,
                               kind="ExternalInput")
    # fusion_w: [p, j, c, m] (bf16, one DMA)
    fusion_w = nc.dram_tensor("fusion_w", [P, DCH, FCH, P], bf16,
                              kind="ExternalInput")
    at_f = nc.dram_tensor("at_f", [P, NCH, N], bf16, kind="ExternalInput")
    at_8 = nc.dram_tensor("at_8", [P, CC, 2, N], fp8, kind="ExternalInput")
    wl8 = nc.dram_tensor("wl8", [L, P, KK, 2, D], fp8, kind="ExternalInput")
    wcn = nc.dram_tensor("wcn", [L, DCH, P, D], f32r, kind="ExternalInput")
    wih8 = nc.dram_tensor("wih8", [P, 2 * DCH, KK, 2, P], fp8,
                          kind="ExternalInput")
    whh8 = nc.dram_tensor("whh8", [P, GCH, KK, 2, P], fp8,
                          kind="ExternalInput")
    # smalls: [bsum(12) | bihn(6) | bhhn_s(6) | fusion_b(6) | mask(4)]
    smalls = nc.dram_tensor("smalls", [P, 34], f32, kind="ExternalInput")
    out = nc.dram_tensor("out", [N, D], f32, kind="ExternalOutput")

    wide = pool_wide
    with tile.TileContext(nc) as tc:
        with (
            tc.tile_pool(name="consts", bufs=1) as consts,
            tc.tile_pool(name="wts", bufs=1) as wts,
            tc.tile_pool(name="wlp", bufs=2) as wlp,
            tc.tile_pool(name="wcnp", bufs=4 if wide else 6) as wcnp,
            tc.tile_pool(name="tokg", bufs=2 if wide else 4) as tokg,
            tc.tile_pool(name="hp", bufs=10 if wide else 12) as hp,
            tc.tile_pool(name="hf8", bufs=6) as hf8,
            tc.tile_pool(name="mnp", bufs=4) as mnp,
            tc.tile_pool(name="mfp", bufs=4 if wide else 5) as mfp,
            tc.tile_pool(name="aggp", bufs=5) as aggp,
            tc.tile_pool(name="gp", bufs=14 if wide else 17) as gp,
            tc.tile_pool(name="obp", bufs=2) as obp,
            tc.tile_pool(name="ps5", bufs=4, space="PSUM") as ps5,
            tc.tile_pool(name="psN", bufs=4, space="PSUM") as psN,
        ):
            # ---- token gather first: it gates the whole front of the kernel
            tok_idx_sb = consts.tile([P, T // 16], i16)
            nc.sync.dma_start(out=tok_idx_sb[:], in_=tok_idx[:])
            if wide:
                pool_sb = consts.tile([P, TCH, N], f32r)
            else:
                pool_sb = consts.tile([P, TCH, BLK], bf16)
            nc.sync.dma_start(out=pool_sb[:], in_=poolm[:])

            tt_sb = consts.tile([TYPES, TD], f32r)
            nc.sync.dma_start(out=tt_sb[:], in_=type_table[:])
            oh_sb = consts.tile([TYPES, N], f32r)
            nc.sync.dma_start(out=oh_sb[:], in_=typ_oh[:])

            gdt = f32r if wide else bf16
            gath = []
            for s in range(GS):
                tg = tokg.tile([P, GC, D], gdt, tag="tokg", name=f"tokg{s}")
                nc.gpsimd.dma_gather(
                    tg[:], word_d[:],
                    tok_idx_sb[:, s * (GT // 16) : (s + 1) * (GT // 16)],
                    GT, GT, D, queue_num=s % 2,
                )
                gath.append(tg)

            # ---- remaining constants / small inputs ----
            identity = consts.tile([P, P], f32)
            make_identity(nc, identity[:])
            smalls_sb = consts.tile([P, 34], f32)
            nc.sync.dma_start(out=smalls_sb[:], in_=smalls[:])
            bsum_sb = smalls_sb[:, 0:12]
            bihn_sb = smalls_sb[:, 12:18]
            bhhn_sb = smalls_sb[:, 18:24]
            fb_sb = smalls_sb[:, 24:30]
            mask_sb = smalls_sb[:, 30:34]

            # weight loads, in the order the DMA pipe must serve them
            # (the cost model serializes all DMA transfers on one pipe)
            fw_sb = wts.tile([P, DCH, FCH, P], bf16)
            nc.scalar.dma_start(out=fw_sb[:], in_=fusion_w[:])
            wl_sb = wlp.tile([P, KK, 2, D], fp8, tag="wlp", name="wl0")
            nc.scalar.dma_start(out=wl_sb[:], in_=wl8[0])
            at8_sb = wts.tile([P, CC, 2, N], fp8)
            nc.scalar.dma_start(out=at8_sb[:], in_=at_8[:])
            whh_sb = wts.tile([P, GCH, KK, 2, P], fp8)
            nc.sync.dma_start(out=whh_sb[:], in_=whh8[:])
            wih_sb = wts.tile([P, 2 * DCH, KK, 2, P], fp8)
            nc.scalar.dma_start(out=wih_sb[:], in_=wih8[:])
            wcn_sb = [
                wcnp.tile([P, D], f32r, tag="wcnp", name=f"wcn0_{k}")
                for k in range(DCH)
            ]
            for k in range(DCH):
                eng = nc.sync if k % 2 else nc.scalar
                eng.dma_start(out=wcn_sb[k][:], in_=wcn[0, k])
            atf_sb = wts.tile([P, NCH, N], bf16)
            nc.sync.dma_start(out=atf_sb[:], in_=at_f[:])

            # fused embedding, feature-major bf16: chunk 0 = type_e,
            # chunks 1..6 = text_e
            fusedT = [
                consts.tile([P, NF], bf16, name=f"fusedT{k}")
                for k in range(FCH)
            ]

            # type_eT = type_table.T @ onehot  (f32r, K=64)
            ptyp = ps5.tile([P, NF], f32, tag="ps5")
            nc.tensor.matmul(
                out=ptyp[:], lhsT=tt_sb[:], rhs=oh_sb[:], start=True, stop=True
            )
            nc.scalar.activation(out=fusedT[0][:], in_=ptyp[:], func=Ident,
                                 bias=0.0)

            # token pooling: 128 tokens -> 32 nodes (block) / 512 (wide)
            for fch in range(DCH):
                pf = ps5.tile([P, NF], f32, tag="ps5", name=f"pf{fch}")
                for c in range(TCH):
                    s, c2 = divmod(c, GC)
                    if wide:
                        nc.tensor.matmul(
                            out=pf[:],
                            lhsT=gath[s][:, c2, fch * P : (fch + 1) * P],
                            rhs=pool_sb[:, c, :],
                            start=(c == 0), stop=(c == TCH - 1),
                        )
                    else:
                        nc.tensor.matmul(
                            out=pf[:, c * BLK : (c + 1) * BLK],
                            lhsT=gath[s][:, c2, fch * P : (fch + 1) * P],
                            rhs=pool_sb[:, c, :],
                            start=True, stop=True,
                        )
                nc.scalar.activation(out=fusedT[1 + fch][:], in_=pf[:],
                                     func=Ident, bias=0.0)

            # ---- fusion matmul: h0 feature-major f32 + fp8 shadow ----
            hT = []
            hF = [
                hf8.tile([P, 2, NF], fp8, tag="hf8", name=f"h0q{kk}")
                for kk in range(KK)
            ]
            for j in range(DCH):
                pfj = ps5.tile([P, NF], f32, tag="ps5")
                for c in range(FCH):
                    nc.tensor.matmul(
                        out=pfj[:],
                        lhsT=fw_sb[:, j, c, :],
                        rhs=fusedT[c][:],
                        start=(c == 0), stop=(c == FCH - 1),
                    )
                hj = hp.tile([P, NF], f32r, tag="hp", name=f"h0_{j}")
                nc.scalar.activation(
                    out=hj[:], in_=pfj[:], func=Ident,
                    bias=fb_sb[:, j : j + 1],
                )
                hT.append(hj)
                nc.vector.tensor_scalar_mul(
                    out=hF[j // 2][:, j % 2, :], in0=hj[:],
                    scalar1=float(2 ** AH),
                )

            # ---- GGNN layers ----
            for l in range(L):
                # m8 = h8 @ Wl8 (fp8 DR) and m' = h @ Wcn (f32r, exact),
                # both node-major, chunk-interleaved so the f32r m' work
                # covers the psum->sbuf copy latency of m8
                mN = [
                    mnp.tile([P, 2, D], fp8, tag="mnp", name=f"mN{l}_{cc}")
                    for cc in range(CC)
                ]
                mpN = [
                    mfp.tile([P, D], bf16, tag="mfp", name=f"mp{l}_{i}")
                    for i in range(NCH)
                ]

                def emit_m8(i):
                    pma = ps5.tile([P, NF], f32, tag="ps5", name=f"pma{l}_{i}")
                    pmb = ps5.tile([P, D - NF], f32, tag="ps5",
                                   name=f"pmb{l}_{i}")
                    for kk in range(KK):
                        lh = hF[kk][:, :, i * P : (i + 1) * P]
                        nc.tensor.matmul(
                            out=pma[:], lhsT=lh, rhs=wl_sb[:, kk, :, :NF],
                            start=(kk == 0), stop=(kk == KK - 1), perf_mode=DR,
                        )
                        nc.tensor.matmul(
                            out=pmb[:], lhsT=lh, rhs=wl_sb[:, kk, :, NF:D],
                            start=(kk == 0), stop=(kk == KK - 1), perf_mode=DR,
                        )
                    nc.scalar.activation(
                        out=mN[i // 2][:, i % 2, :NF], in_=pma[:],
                        func=Ident, bias=0.0,
                    )
                    nc.vector.tensor_copy(
                        out=mN[i // 2][:, i % 2, NF:D], in_=pmb[:]
                    )

                def emit_mp(i):
                    pca = ps5.tile([P, NF], f32, tag="ps5", name=f"pca{l}_{i}")
                    pcb = ps5.tile([P, D - NF], f32, tag="ps5",
                                   name=f"pcb{l}_{i}")
                    for k in range(DCH):
                        lh = hT[k][:, i * P : (i + 1) * P]
                        nc.tensor.matmul(
                            out=pca[:], lhsT=lh, rhs=wcn_sb[k][:, :NF],
                            start=(k == 0), stop=(k == DCH - 1),
                        )
                        nc.tensor.matmul(
                            out=pcb[:], lhsT=lh, rhs=wcn_sb[k][:, NF:D],
                            start=(k == 0), stop=(k == DCH - 1),
                        )
                    nc.scalar.activation(out=mpN[i][:, :NF], in_=pca[:],
                                         func=Ident, bias=0.0)
                    nc.vector.tensor_copy(out=mpN[i][:, NF:D], in_=pcb[:])

                # m' first: it reads the f32 master h directly (no
                # dependence on the fp8 quantize tail of the previous layer);
                # contract k=0..3 first so the PE has work before the last
                # h chunks of the previous layer finish
                mp_ps = {}
                for i in range(NCH):
                    pca = ps5.tile([P, NF], f32, tag="ps5",
                                   name=f"pca{l}_{i}")
                    pcb = ps5.tile([P, D - NF], f32, tag="ps5",
                                   name=f"pcb{l}_{i}")
                    mp_ps[i] = (pca, pcb)
                    for k in range(4):
                        lh = hT[k][:, i * P : (i + 1) * P]
                        nc.tensor.matmul(
                            out=pca[:], lhsT=lh, rhs=wcn_sb[k][:, :NF],
                            start=(k == 0), stop=False,
                        )
                        nc.tensor.matmul(
                            out=pcb[:], lhsT=lh, rhs=wcn_sb[k][:, NF:D],
                            start=(k == 0), stop=False,
                        )
                    if i % 2 == 1:
                        for j in (i - 1, i):
                            pca_, pcb_ = mp_ps[j]
                            for k in range(4, DCH):
                                lh = hT[k][:, j * P : (j + 1) * P]
                                nc.tensor.matmul(
                                    out=pca_[:], lhsT=lh,
                                    rhs=wcn_sb[k][:, :NF],
                                    start=False, stop=(k == DCH - 1),
                                )
                                nc.tensor.matmul(
                                    out=pcb_[:], lhsT=lh,
                                    rhs=wcn_sb[k][:, NF:D],
                                    start=False, stop=(k == DCH - 1),
                                )
                            nc.scalar.activation(out=mpN[j][:, :NF],
                                                 in_=pca_[:], func=Ident,
                                                 bias=0.0)
                            nc.vector.tensor_copy(out=mpN[j][:, NF:D],
                                                  in_=pcb_[:])
                for i in range(NCH):
                    emit_m8(i)

                # pre-open r/z psum groups with their gh contributions so
                # the PE has work while agg8 quantizes drain
                rz_ps = {}

                def emit_gh_rz(i):
                    prs = []
                    for g in (i, DCH + i):
                        pg = ps5.tile([P, NF], f32, tag="ps5",
                                      name=f"prz{l}_{g}")
                        for kk in range(KK):
                            nc.tensor.matmul(
                                out=pg[:], lhsT=whh_sb[:, g, kk, :, :],
                                rhs=hF[kk][:],
                                start=(kk == 0), stop=False, perf_mode=DR,
                            )
                        prs.append(pg)
                    rz_ps[i] = prs

                emit_gh_rz(0)

                # agg8 = A8-contraction of m8 (fp8 DR), feature-major
                aggF = [
                    aggp.tile([P, 2, NF], fp8, tag="aggp", name=f"agg{l}_{kk}")
                    for kk in range(KK)
                ]
                for fch in range(DCH):
                    pag = ps5.tile([P, NF], f32, tag="ps5")
                    for cc in range(CC):
                        nc.tensor.matmul(
                            out=pag[:],
                            lhsT=mN[cc][:, :, fch * P : (fch + 1) * P],
                            rhs=at8_sb[:, cc, :, :],
                            start=(cc == 0), stop=(cc == CC - 1), perf_mode=DR,
                        )
                    nc.scalar.activation(
                        out=aggF[fch // 2][:, fch % 2, :], in_=pag[:],
                        func=Ident, bias=0.0, scale=float(2.0 ** (AG - AM)),
                    )

                # prefetch next layer's weights while gates run
                if l + 1 < L:
                    wl_next = wlp.tile([P, KK, 2, D], fp8, tag="wlp",
                                       name=f"wl{l + 1}")
                    nc.scalar.dma_start(out=wl_next[:], in_=wl8[l + 1])
                    wcn_next = [
                        wcnp.tile([P, D], f32r, tag="wcnp",
                                  name=f"wcn{l + 1}_{k}")
                        for k in range(DCH)
                    ]
                    for k in range(DCH):
                        eng = nc.sync if k % 2 else nc.scalar
                        eng.dma_start(out=wcn_next[k][:], in_=wcn[l + 1, k])

                # GRU gates, 128 gate rows at a time (feature chunk i)
                hTn = []
                hFn = [
                    hf8.tile([P, 2, NF], fp8, tag="hf8",
                             name=f"h{l + 1}q{kk}")
                    for kk in range(KK)
                ] if l + 1 < L else []
                # stage-skewed gate pipeline: A(i) = matmuls + sigmoids,
                # B(i) = rn/tn/tanh, C(i) = h' update + fp8 shadow.  Skewing
                # keeps every engine's in-order queue dependency-ready.
                st = [dict() for _ in range(DCH)]

                def stageA(i):
                    rz = []
                    for gi_, (g, pg) in enumerate(
                        ((i, rz_ps[i][0]), (DCH + i, rz_ps[i][1]))
                    ):
                        for kk in range(KK):
                            nc.tensor.matmul(
                                out=pg[:], lhsT=wih_sb[:, g, kk, :, :],
                                rhs=aggF[kk][:],
                                start=False, stop=(kk == KK - 1), perf_mode=DR,
                            )
                        gs = gp.tile([P, NF], f32, tag="gp",
                                     name=f"g{l}_{i}_{gi_}")
                        nc.scalar.activation(
                            out=gs[:], in_=pg[:], func=Sigmoid,
                            bias=bsum_sb[:, g : g + 1],
                            scale=float(2.0 ** -SG),
                        )
                        rz.append(gs)
                    del rz_ps[i]
                    if i + 1 < DCH and i + 1 not in rz_ps:
                        emit_gh_rz(i + 1)
                    st[i]["r"], st[i]["z"] = rz

                    g = 2 * DCH + i
                    pghn = psN.tile([P, NF], f32, tag="psN",
                                    name=f"pghn{l}_{i}")
                    for kk in range(KK):
                        nc.tensor.matmul(
                            out=pghn[:], lhsT=whh_sb[:, g, kk, :, :],
                            rhs=hF[kk][:],
                            start=(kk == 0), stop=(kk == KK - 1), perf_mode=DR,
                        )
                    pgin = psN.tile([P, NF], f32, tag="psN",
                                    name=f"pgin{l}_{i}")
                    for c in range(NCH):
                        nc.tensor.matmul(
                            out=pgin[:],
                            lhsT=mpN[c][:, i * P : (i + 1) * P],
                            rhs=atf_sb[:, c, :],
                            start=(c == 0), stop=(c == NCH - 1),
                        )
                    st[i]["pghn"], st[i]["pgin"] = pghn, pgin

                def stageB(i):
                    s_i = st[i]
                    rn = gp.tile([P, NF], f32, tag="gp", name=f"rn{l}_{i}")
                    if has_bhhn:
                        hb = gp.tile([P, NF], f32, tag="gp",
                                     name=f"hb{l}_{i}")
                        nc.scalar.activation(
                            out=hb[:], in_=s_i["pghn"][:], func=Ident,
                            bias=bhhn_sb[:, i : i + 1], scale=1.0,
                        )
                        nc.vector.tensor_mul(out=rn[:], in0=s_i["r"][:],
                                             in1=hb[:])
                    else:
                        nc.vector.tensor_mul(out=rn[:], in0=s_i["r"][:],
                                             in1=s_i["pghn"][:])
                    tn = gp.tile([P, NF], f32, tag="gp", name=f"tn{l}_{i}")
                    nc.vector.tensor_add(out=tn[:], in0=s_i["pgin"][:],
                                         in1=rn[:])
                    nn_ = gp.tile([P, NF], f32, tag="gp", name=f"nn{l}_{i}")
                    nc.scalar.activation(
                        out=nn_[:], in_=tn[:], func=Tanh,
                        bias=bihn_sb[:, i : i + 1], scale=float(2.0 ** -SG),
                    )
                    s_i["n"] = nn_

                def stageC(i):
                    s_i = st[i]
                    nn_ = s_i["n"]
                    s_ = gp.tile([P, NF], f32, tag="gp", name=f"s{l}_{i}")
                    nc.gpsimd.tensor_sub(out=s_[:], in0=hT[i][:], in1=nn_[:])
                    sz = gp.tile([P, NF], f32, tag="gp", name=f"sz{l}_{i}")
                    nc.gpsimd.tensor_mul(out=sz[:], in0=s_i["z"][:], in1=s_[:])
                    hj = hp.tile([P, NF], f32r, tag="hp",
                                 name=f"h{l + 1}_{i}")
                    nc.vector.tensor_add(out=hj[:], in0=nn_[:], in1=sz[:])
                    hTn.append(hj)
                    if l + 1 < L:
                        nc.vector.tensor_scalar_mul(
                            out=hFn[i // 2][:, i % 2, :], in0=hj[:],
                            scalar1=float(2 ** AH),
                        )

                for t in range(DCH + 2):
                    if t < DCH:
                        stageA(t)
                    if 1 <= t + 0 and 0 <= t - 1 < DCH:
                        stageB(t - 1)
                    if 0 <= t - 2 < DCH:
                        stageC(t - 2)
                hT, hF = hTn, hFn
                if l + 1 < L:
                    wl_sb = wl_next
                    wcn_sb = wcn_next

            # ---- transpose back to node-major, mask, write out ----
            # j-major: transposes for feature chunk j start as soon as the
            # last layer's h'(j) lands, overlapping the pointwise drain
            poas = [ps5.tile([P, NF], f32, tag="ps5", name=f"poa{i}")
                    for i in range(NCH)]
            pobs = [psN.tile([P, D - NF], f32, tag="psN", name=f"pob{i}")
                    for i in range(NCH)]
            for j in range(DCH):
                for i in range(NCH):
                    dst = poas[i][:, j * P : (j + 1) * P] if j < 4 else \
                        pobs[i][:, (j - 4) * P : (j - 3) * P]
                    nc.tensor.transpose(
                        out=dst,
                        in_=hT[j][:, i * P : (i + 1) * P].bitcast(f32),
                        identity=identity[:],
                    )
            for i in range(NCH):
                ob = obp.tile([P, D], f32, tag="obp")
                nc.vector.tensor_scalar_mul(
                    out=ob[:, :NF], in0=poas[i][:],
                    scalar1=mask_sb[:, i : i + 1],
                )
                nc.scalar.activation(
                    out=ob[:, NF:D], in_=pobs[i][:], func=Ident, bias=0.0,
                    scale=mask_sb[:, i : i + 1],
                )
                eng = nc.sync if i % 2 else nc.scalar
                eng.dma_start(out=out[i * P : (i + 1) * P, :NF],
                              in_=ob[:, :NF])
                eng2 = nc.scalar if i % 2 else nc.sync
                eng2.dma_start(out=out[i * P : (i + 1) * P, NF:D],
                               in_=ob[:, NF:D])

    nc.compile()
    return nc


@functools.lru_cache(maxsize=4)
def _get_nc(pool_wide: bool, has_bhhn: bool) -> bass.Bass:
    return build_nc(pool_wide, has_bhhn)


def _prep_shared(inputs, pool_wide):
    """Weight tensors identical across graphs, pre-quantized / pre-laid-out
    partition-major so every DMA is contiguous per partition."""
    word = np.asarray(inputs["word_emb"], np.float32)
    tt = np.ascontiguousarray(np.asarray(inputs["type_table"], np.float32))
    fw = np.asarray(inputs["fusion_w"], np.float32)          # [F, D]
    # [p, j, c, m] = fw[c*128+p, j*128+m]
    fusion_w = np.ascontiguousarray(
        fw.reshape(FCH, P, DCH, P).transpose(1, 2, 0, 3)
    ).astype(NPBF)
    wl_w = np.asarray(inputs["ggnn_w"], np.float32)          # [L, D, D]
    wl8 = np.ascontiguousarray(
        (wl_w * 2.0 ** BWL).reshape(L, KK, 2, P, D).transpose(0, 3, 1, 2, 4)
    ).astype(NPF8)
    wih_w = np.asarray(inputs["gru_w_ih"], np.float32)       # [3D, D]
    whh_w = np.asarray(inputs["gru_w_hh"], np.float32)
    # [p, g, kk, i, m] = W.T[(2kk+i)*128+p, g*128+m] * scale
    wih8 = np.ascontiguousarray(
        (wih_w[: 2 * D].T * 2.0 ** BIH)
        .reshape(KK, 2, P, 2 * DCH, P).transpose(2, 3, 0, 1, 4)
    ).astype(NPF8)
    whh8 = np.ascontiguousarray(
        (whh_w.T * 2.0 ** BHH)
        .reshape(KK, 2, P, GCH, P).transpose(2, 3, 0, 1, 4)
    ).astype(NPF8)
    # Wcn_l = Wl @ Wih_n^T, pre-scaled by 2^SG; [l, k, p, j]
    wih_n = wih_w[2 * D :]
    wcn = np.stack([
        (wl_w[l].astype(np.float64) @ wih_n.T.astype(np.float64))
        for l in range(L)
    ]).astype(np.float32) * 2.0 ** SG
    wcn = np.ascontiguousarray(wcn.reshape(L, DCH, P, D))
    bih = np.asarray(inputs["gru_b_ih"], np.float32)
    bhh = np.asarray(inputs["gru_b_hh"], np.float32)
    smalls = np.zeros((P, 34), np.float32)
    smalls[:, 0:12] = (bih[: 2 * D] + bhh[: 2 * D]).reshape(2 * DCH, P).T
    smalls[:, 12:18] = bih[2 * D :].reshape(DCH, P).T
    smalls[:, 18:24] = (bhh[2 * D :] * 2.0 ** SG).reshape(DCH, P).T
    smalls[:, 24:30] = (
        np.asarray(inputs["fusion_b"], np.float32).reshape(DCH, P).T
    )
    shared = dict(
        type_table=tt, fusion_w=fusion_w, wl8=wl8, wcn=wcn, wih8=wih8,
        whh8=whh8, _smalls_base=smalls,
    )
    if pool_wide:
        shared["word_f32"] = np.ascontiguousarray(word)
    else:
        shared["word_bf"] = np.ascontiguousarray(word.astype(NPBF))
    return shared


def _graph_blockable(inputs, b):
    seg = np.asarray(inputs["token_seg_ids"][b], np.int64)
    tcol = np.arange(T) // P
    return bool(np.all((seg >= tcol * BLK) & (seg < (tcol + 1) * BLK)))


def _prep_graph(inputs, b, pool_wide):
    tok = np.asarray(inputs["node_token_ids"][b], np.int64)
    typ = np.asarray(inputs["node_types"][b], np.int32)
    seg = np.asarray(inputs["token_seg_ids"][b], np.int64)
    lens = np.asarray(inputs["node_token_lens"][b], np.float64)
    glen = int(np.asarray(inputs["graph_node_lens"][b]))
    esrc = np.asarray(inputs["edge_src"][b], np.int64)
    edst = np.asarray(inputs["edge_dst"][b], np.int64)
    ew = np.asarray(inputs["edge_weight"][b], np.float32)

    # token idxs for dma_gather: GS splits of GT idxs, each wrapped into
    # 16 partitions and replicated to 128 partitions
    tok16 = tok.astype(np.int16)
    cols = []
    for s in range(GS):
        w16 = tok16[s * GT : (s + 1) * GT].reshape(GT // 16, 16).T
        cols.append(np.tile(w16, (8, 1)))
    tok_idx = np.ascontiguousarray(np.concatenate(cols, axis=1))

    typ_oh = np.zeros((TYPES, N), np.float32)
    typ_oh[typ, np.arange(N)] = 1.0

    # dense adjacency A[src, dst]: f32r copy + DoubleRow-paired fp8 copy
    at = np.zeros((N, N), np.float32)
    np.add.at(at, (esrc, edst), ew)
    at_f = np.ascontiguousarray(
        at.reshape(NCH, P, N).transpose(1, 0, 2)
    ).astype(NPBF)
    at_8 = np.ascontiguousarray(
        at.reshape(CC, 2, P, N).transpose(2, 0, 1, 3)
    ).astype(NPF8)

    winv = np.zeros(N, np.float64)
    nzmask = lens != 0
    winv[nzmask] = 1.0 / lens[nzmask]
    t_ = np.arange(T)
    c_ = t_ // P
    if pool_wide:
        pm = np.zeros((TCH, P, N), np.float32)
        pm[c_, t_ % P, seg] = winv[seg]
        pool_arr = np.ascontiguousarray(pm.transpose(1, 0, 2))
        pool_key = "poolw"
    else:
        pm = np.zeros((TCH, P, BLK), np.float32)
        pm[c_, t_ % P, seg - c_ * BLK] = winv[seg]
        pool_arr = np.ascontiguousarray(pm.transpose(1, 0, 2)).astype(NPBF)
        pool_key = "poolm"

    keep = min(glen, MAX_NODE_LEN)
    mask = np.ascontiguousarray(
        (np.arange(N) < keep).astype(np.float32).reshape(NCH, P).T
    )
    return {
        "tok_idx": tok_idx, "typ_oh": typ_oh, "at_f": at_f, "at_8": at_8,
        pool_key: pool_arr, "_mask": mask,
    }


def kernel(**inputs) -> np.ndarray:
    pool_wide = not all(_graph_blockable(inputs, b) for b in range(B))
    has_bhhn = bool(
        np.any(np.asarray(inputs["gru_b_hh"], np.float32)[2 * D :] != 0.0)
    )
    shared = _prep_shared(inputs, pool_wide)
    smalls_base = shared.pop("_smalls_base")
    per_graph = [_prep_graph(inputs, b, pool_wide) for b in range(B)]
    in_maps = []
    for b in range(B):
        g = dict(per_graph[b])
        sm = smalls_base.copy()
        sm[:, 30:34] = g.pop("_mask")
        g["smalls"] = sm
        in_maps.append({**shared, **g})
    nc = _get_nc(pool_wide, has_bhhn)
    res = bass_utils.run_bass_kernel_spmd(nc, in_maps, core_ids=list(range(B)))
    global _last_exec_ns
    _last_exec_ns = res.exec_time_ns
    out = np.stack([r["out"] for r in res.results]).astype(np.float32)
    return out


_last_exec_ns = None
_last_nc = None


# revision 4
# speedup vs baseline: 1.7199x; 1.0382x over previous
# GGNN encoder kernel for Trainium2 (Bass/Tile), data-parallel over the
# batch dimension: 8 graphs -> 8 NeuronCores, one graph per core.
#
# Mixed-precision design ("R2"):
#  - r/z gates and gh_n run as fp8(e4m3) DoubleRow matmuls (two 128-deep
#    K-planes per instruction at 0.5 cycles/row).  Their quantization
#    noise is squashed by the sigmoid (and by r*gh_n being small).
#  - The sensitive n-gate input gi_n = A^T (h @ Wl @ Wih_n^T) is computed
#    exactly in f32r via a host-side weight fold (Wcn = Wl @ Wih_n^T),
#    which also deletes the separate per-layer m matmul from this path.
#  - h master stays fp32; embeddings/pooling/fusion run in bf16/f32r.
#  - All fp8 operands carry power-of-two scales, folded exactly into the
#    activation-function scale arguments.
#
# Per-core computation (one graph):
#   type_e  = type_table[node_types]                       # f32r one-hot matmul
#   tok_e   = word_emb[node_token_ids]                     # bf16 SWDGE dma_gather
#   text_e  = segment_mean(tok_e, token_seg_ids)           # bf16 pooling matmul
#   h       = concat(type_e, text_e) @ fusion_w + b        # bf16 matmul
#   4 x GGNN layer:
#     m8   = h8 @ Wl8          (fp8 DR)      -> agg8 = A8-contract (fp8 DR)
#     r,z  = sigmoid(Wih8 agg8 + Whh8 h8 + b)              (fp8 DR psum)
#     gin  = A-contract(h @ Wcn)                           (f32r, exact)
#     ghn  = Whh_n8 h8                                     (fp8 DR)
#     n    = tanh(gin + r*ghn + b);  h' = n + z*(h - n)    (fp32 pointwise)
#   out     = mask * h                                     # PE transpose epilogue

import functools

import numpy as np
import ml_dtypes

import concourse.bass as bass
import concourse.mybir as mybir
import concourse.tile as tile
from concourse import bacc, bass_utils
from concourse.masks import make_identity

# Problem shapes (hardcoded: kernel must be self-contained).
B, N, T, D, TD, L = 8, 512, 2048, 768, 128, 4
V, TYPES = 30522, 64
MAX_NODE_LEN = 512
F = TD + D            # 896 fused embedding dim
P = 128               # partitions
DCH = D // P          # 6 feature chunks
KK = DCH // 2         # 3 feature chunk pairs (DoubleRow planes)
NCH = N // P          # 4 node chunks
CC = NCH // 2         # 2 node chunk pairs
FCH = F // P          # 7 fused chunks
GCH = 3 * DCH         # 18 gate row chunks
TCH = T // P          # 16 token chunks
BLK = N // TCH        # 32 nodes per token chunk (block-pooling case)
NF = 512              # free-dim tile (nodes)
GS = 4                # token gather splits
GT = T // GS          # tokens per gather split (512)
GC = GT // P          # 128-chunks per gather split (4)

# power-of-two scale exponents for the fp8 operands
AH = 4                # h -> fp8
AM = 9                # m -> fp8
AG = 4                # agg -> fp8
BWL = 5               # ggnn_w
BIH = 5               # gru_w_ih (r,z rows)
BHH = 5               # gru_w_hh
SG = BIH + AG         # gate psum scale (== BHH + AH); Wcn also pre-scaled 2^SG
assert SG == BHH + AH

f32 = mybir.dt.float32
f32r = mybir.dt.float32r
bf16 = mybir.dt.bfloat16
fp8 = mybir.dt.float8e4
i16 = mybir.dt.int16
NPF8 = ml_dtypes.float8_e4m3
NPBF = ml_dtypes.bfloat16

Sigmoid = mybir.ActivationFunctionType.Sigmoid
Tanh = mybir.ActivationFunctionType.Tanh
Ident = mybir.ActivationFunctionType.Identity
DR = mybir.MatmulPerfMode.DoubleRow


def build_nc(pool_wide: bool, has_bhhn: bool) -> bass.Bass:
    nc = bacc.Bacc(num_swdge_queues=2, dynamic_dma_scratch_size=32768)

    tok_idx = nc.dram_tensor("tok_idx", [P, T // 16], i16, kind="ExternalInput")
    typ_oh = nc.dram_tensor("typ_oh", [TYPES, N], f32r, kind="ExternalInput")
    type_table = nc.dram_tensor("type_table", [TYPES, TD], f32r,
                                kind="ExternalInput")
    word_d = nc.dram_tensor("word_bf", [V, D], bf16, kind="ExternalInput")
    if pool_wide:
        poolm = nc.dram_tensor("poolw", [P, TCH, N], bf16, kind="ExternalInput")
    else:
        poolm = nc.dram_tensor("poolm", [P, TCH, BLK], bf16,
                               kind="ExternalInput")
    # fusion_w: [p, j, c, m] (bf16, one DMA)
    fusion_w = nc.dram_tensor("fusion_w", [P, DCH, FCH, P], bf16,
                              kind="ExternalInput")
    at_f = nc.dram_tensor("at_f", [P, NCH, N], bf16, kind="ExternalInput")
    at_8 = nc.dram_tensor("at_8", [P, CC, 2, N], fp8, kind="ExternalInput")
    wl8 = nc.dram_tensor("wl8", [L, P, KK, 2, D], fp8, kind="ExternalInput")
    wcn = nc.dram_tensor("wcn", [L, DCH, P, D], f32r, kind="ExternalInput")
    wih8 = nc.dram_tensor("wih8", [P, 2 * DCH, KK, 2, P], fp8,
                          kind="ExternalInput")
    whh8 = nc.dram_tensor("whh8", [P, GCH, KK, 2, P], fp8,
                          kind="ExternalInput")
    # smalls: [bsum(12) | bihn(6) | bhhn_s(6) | fusion_b(6) | mask(4)]
    smalls = nc.dram_tensor("smalls", [P, 34], f32, kind="ExternalInput")
    out = nc.dram_tensor("out", [N, D], f32, kind="ExternalOutput")

    wide = pool_wide
    with tile.TileContext(nc) as tc:
        with (
            tc.tile_pool(name="consts", bufs=1) as consts,
            tc.tile_pool(name="wts", bufs=1) as wts,
            tc.tile_pool(name="wlp", bufs=2) as wlp,
            tc.tile_pool(name="wcnp", bufs=6) as wcnp,
            tc.tile_pool(name="tokg", bufs=4) as tokg,
            tc.tile_pool(name="hp", bufs=12) as hp,
            tc.tile_pool(name="hf8", bufs=5 if wide else 6) as hf8,
            tc.tile_pool(name="mnp", bufs=4) as mnp,
            tc.tile_pool(name="mfp", bufs=5) as mfp,
            tc.tile_pool(name="aggp", bufs=4 if wide else 5) as aggp,
            tc.tile_pool(name="gp", bufs=11 if wide else 17) as gp,
            tc.tile_pool(name="obp", bufs=2) as obp,
            tc.tile_pool(name="ps5", bufs=6, space="PSUM") as ps5,
            tc.tile_pool(name="psN", bufs=2, space="PSUM") as psN,
        ):
            # ---- token gather first: it gates the whole front of the kernel
            tok_idx_sb = consts.tile([P, T // 16], i16)
            nc.sync.dma_start(out=tok_idx_sb[:], in_=tok_idx[:])
            if wide:
                pool_sb = consts.tile([P, TCH, N], bf16)
            else:
                pool_sb = consts.tile([P, TCH, BLK], bf16)
            nc.sync.dma_start(out=pool_sb[:], in_=poolm[:])

            tt_sb = consts.tile([TYPES, TD], f32r)
            nc.sync.dma_start(out=tt_sb[:], in_=type_table[:])
            oh_sb = consts.tile([TYPES, N], f32r)
            nc.sync.dma_start(out=oh_sb[:], in_=typ_oh[:])

            # fusion weights ahead of the token gathers on the DMA pipe:
            # fusion is the first consumer after pooling
            fw_sb = wts.tile([P, DCH, FCH, P], bf16)
            nc.scalar.dma_start(out=fw_sb[:], in_=fusion_w[:])
            wl_sb = wlp.tile([P, KK, 2, D], fp8, tag="wlp", name="wl0")
            nc.scalar.dma_start(out=wl_sb[:], in_=wl8[0])
            at8_sb = wts.tile([P, CC, 2, N], fp8)
            nc.scalar.dma_start(out=at8_sb[:], in_=at_8[:])

            gdt = bf16
            gath = []
            for s in range(GS):
                tg = tokg.tile([P, GC, D], gdt, tag="tokg", name=f"tokg{s}")
                nc.gpsimd.dma_gather(
                    tg[:], word_d[:],
                    tok_idx_sb[:, s * (GT // 16) : (s + 1) * (GT // 16)],
                    GT, GT, D, queue_num=s % 2,
                )
                gath.append(tg)

            # ---- remaining constants / small inputs ----
            identity = consts.tile([P, P], f32)
            make_identity(nc, identity[:])
            smalls_sb = consts.tile([P, 34], f32)
            nc.sync.dma_start(out=smalls_sb[:], in_=smalls[:])
            bsum_sb = smalls_sb[:, 0:12]
            bihn_sb = smalls_sb[:, 12:18]
            bhhn_sb = smalls_sb[:, 18:24]
            fb_sb = smalls_sb[:, 24:30]
            mask_sb = smalls_sb[:, 30:34]

            # remaining weights in the order the DMA pipe must serve them
            whh_sb = wts.tile([P, GCH, KK, 2, P], fp8)
            nc.sync.dma_start(out=whh_sb[:], in_=whh8[:])
            wih_sb = wts.tile([P, 2 * DCH, KK, 2, P], fp8)
            nc.scalar.dma_start(out=wih_sb[:], in_=wih8[:])
            wcn_sb = [
                wcnp.tile([P, D], f32r, tag="wcnp", name=f"wcn0_{k}")
                for k in range(DCH)
            ]
            for k in range(DCH):
                eng = nc.sync if k % 2 else nc.scalar
                eng.dma_start(out=wcn_sb[k][:], in_=wcn[0, k])
            atf_sb = wts.tile([P, NCH, N], bf16)
            nc.sync.dma_start(out=atf_sb[:], in_=at_f[:])

            # fused embedding, feature-major bf16: chunk 0 = type_e,
            # chunks 1..6 = text_e
            fusedT = [
                consts.tile([P, NF], bf16, name=f"fusedT{k}")
                for k in range(FCH)
            ]

            # type_eT = type_table.T @ onehot  (f32r, K=64)
            ptyp = ps5.tile([P, NF], f32, tag="ps5")
            nc.tensor.matmul(
                out=ptyp[:], lhsT=tt_sb[:], rhs=oh_sb[:], start=True, stop=True
            )
            nc.scalar.activation(out=fusedT[0][:], in_=ptyp[:], func=Ident,
                                 bias=0.0)

            # token pooling: 128 tokens -> 32 nodes (block) / 512 (wide)
            if wide:
                # c-major accumulation: each gather split is consumed once
                # and released (tokg ring holds only 2 splits in wide mode)
                pfs = [
                    ps5.tile([P, NF], f32, tag="ps5", name=f"pf{fch}")
                    for fch in range(DCH)
                ]
                for c in range(TCH):
                    s, c2 = divmod(c, GC)
                    for fch in range(DCH):
                        nc.tensor.matmul(
                            out=pfs[fch][:],
                            lhsT=gath[s][:, c2, fch * P : (fch + 1) * P],
                            rhs=pool_sb[:, c, :],
                            start=(c == 0), stop=(c == TCH - 1),
                        )
                for fch in range(DCH):
                    nc.scalar.activation(out=fusedT[1 + fch][:],
                                         in_=pfs[fch][:], func=Ident,
                                         bias=0.0)
            else:
                for fch in range(DCH):
                    pf = ps5.tile([P, NF], f32, tag="ps5", name=f"pf{fch}")
                    for c in range(TCH):
                        s, c2 = divmod(c, GC)
                        nc.tensor.matmul(
                            out=pf[:, c * BLK : (c + 1) * BLK],
                            lhsT=gath[s][:, c2, fch * P : (fch + 1) * P],
                            rhs=pool_sb[:, c, :],
                            start=True, stop=True,
                        )
                    nc.scalar.activation(out=fusedT[1 + fch][:], in_=pf[:],
                                         func=Ident, bias=0.0)

            # ---- fusion matmul: h0 feature-major f32 + fp8 shadow ----
            hT = []
            hF = [
                hf8.tile([P, 2, NF], fp8, tag="hf8", name=f"h0q{kk}")
                for kk in range(KK)
            ]
            for j in range(DCH):
                pfj = ps5.tile([P, NF], f32, tag="ps5")
                for c in range(FCH):
                    nc.tensor.matmul(
                        out=pfj[:],
                        lhsT=fw_sb[:, j, c, :],
                        rhs=fusedT[c][:],
                        start=(c == 0), stop=(c == FCH - 1),
                    )
                hj = hp.tile([P, NF], f32r, tag="hp", name=f"h0_{j}")
                nc.scalar.activation(
                    out=hj[:], in_=pfj[:], func=Ident,
                    bias=fb_sb[:, j : j + 1],
                )
                hT.append(hj)
                nc.vector.tensor_scalar_mul(
                    out=hF[j // 2][:, j % 2, :], in0=hj[:],
                    scalar1=float(2 ** AH),
                )

            # ---- GGNN layers ----
            for l in range(L):
                # m8 = h8 @ Wl8 (fp8 DR) and m' = h @ Wcn (f32r, exact),
                # both node-major, chunk-interleaved so the f32r m' work
                # covers the psum->sbuf copy latency of m8
                mN = [
                    mnp.tile([P, 2, D], fp8, tag="mnp", name=f"mN{l}_{cc}")
                    for cc in range(CC)
                ]
                mpN = [
                    mfp.tile([P, D], bf16, tag="mfp", name=f"mp{l}_{i}")
                    for i in range(NCH)
                ]

                def emit_m8(i):
                    pma = ps5.tile([P, NF], f32, tag="ps5", name=f"pma{l}_{i}")
                    pmb = ps5.tile([P, D - NF], f32, tag="ps5",
                                   name=f"pmb{l}_{i}")
                    for kk in range(KK):
                        lh = hF[kk][:, :, i * P : (i + 1) * P]
                        nc.tensor.matmul(
                            out=pma[:], lhsT=lh, rhs=wl_sb[:, kk, :, :NF],
                            start=(kk == 0), stop=(kk == KK - 1), perf_mode=DR,
                        )
                        nc.tensor.matmul(
                            out=pmb[:], lhsT=lh, rhs=wl_sb[:, kk, :, NF:D],
                            start=(kk == 0), stop=(kk == KK - 1), perf_mode=DR,
                        )
                    nc.scalar.activation(
                        out=mN[i // 2][:, i % 2, :NF], in_=pma[:],
                        func=Ident, bias=0.0,
                    )
                    nc.vector.tensor_copy(
                        out=mN[i // 2][:, i % 2, NF:D], in_=pmb[:]
                    )

                def emit_mp(i):
                    pca = ps5.tile([P, NF], f32, tag="ps5", name=f"pca{l}_{i}")
                    pcb = ps5.tile([P, D - NF], f32, tag="ps5",
                                   name=f"pcb{l}_{i}")
                    for k in range(DCH):
                        lh = hT[k][:, i * P : (i + 1) * P]
                        nc.tensor.matmul(
                            out=pca[:], lhsT=lh, rhs=wcn_sb[k][:, :NF],
                            start=(k == 0), stop=(k == DCH - 1),
                        )
                        nc.tensor.matmul(
                            out=pcb[:], lhsT=lh, rhs=wcn_sb[k][:, NF:D],
                            start=(k == 0), stop=(k == DCH - 1),
                        )
                    nc.scalar.activation(out=mpN[i][:, :NF], in_=pca[:],
                                         func=Ident, bias=0.0)
                    nc.vector.tensor_copy(out=mpN[i][:, NF:D], in_=pcb[:])

                # m' first: it reads the f32 master h directly (no
                # dependence on the fp8 quantize tail of the previous layer);
                # contract k=0..3 first so the PE has work before the last
                # h chunks of the previous layer finish
                mp_ps = {}
                for i in range(NCH):
                    pca = ps5.tile([P, NF], f32, tag="ps5",
                                   name=f"pca{l}_{i}")
                    pcb = ps5.tile([P, D - NF], f32, tag="ps5",
                                   name=f"pcb{l}_{i}")
                    mp_ps[i] = (pca, pcb)
                    for k in range(4):
                        lh = hT[k][:, i * P : (i + 1) * P]
                        nc.tensor.matmul(
                            out=pca[:], lhsT=lh, rhs=wcn_sb[k][:, :NF],
                            start=(k == 0), stop=False,
                        )
                        nc.tensor.matmul(
                            out=pcb[:], lhsT=lh, rhs=wcn_sb[k][:, NF:D],
                            start=(k == 0), stop=False,
                        )
                    if i % 2 == 1:
                        for j in (i - 1, i):
                            pca_, pcb_ = mp_ps[j]
                            for k in range(4, DCH):
                                lh = hT[k][:, j * P : (j + 1) * P]
                                nc.tensor.matmul(
                                    out=pca_[:], lhsT=lh,
                                    rhs=wcn_sb[k][:, :NF],
                                    start=False, stop=(k == DCH - 1),
                                )
                                nc.tensor.matmul(
                                    out=pcb_[:], lhsT=lh,
                                    rhs=wcn_sb[k][:, NF:D],
                                    start=False, stop=(k == DCH - 1),
                                )
                            nc.scalar.activation(out=mpN[j][:, :NF],
                                                 in_=pca_[:], func=Ident,
                                                 bias=0.0)
                            nc.vector.tensor_copy(out=mpN[j][:, NF:D],
                                                  in_=pcb_[:])
                for i in range(NCH):
                    emit_m8(i)

                # pre-open r/z psum groups with their gh contributions so
                # the PE has work while agg8 quantizes drain
                rz_ps = {}

                def emit_gh_rz(i):
                    prs = []
                    for g in (i, DCH + i):
                        pg = ps5.tile([P, NF], f32, tag="ps5",
                                      name=f"prz{l}_{g}")
                        for kk in range(KK):
                            nc.tensor.matmul(
                                out=pg[:], lhsT=whh_sb[:, g, kk, :, :],
                                rhs=hF[kk][:],
                                start=(kk == 0), stop=False, perf_mode=DR,
                            )
                        prs.append(pg)
                    rz_ps[i] = prs

                emit_gh_rz(0)

                # agg8 = A8-contraction of m8 (fp8 DR), feature-major
                aggF = [
                    aggp.tile([P, 2, NF], fp8, tag="aggp", name=f"agg{l}_{kk}")
                    for kk in range(KK)
                ]
                for fch in range(DCH):
                    pag = ps5.tile([P, NF], f32, tag="ps5")
                    for cc in range(CC):
                        nc.tensor.matmul(
                            out=pag[:],
                            lhsT=mN[cc][:, :, fch * P : (fch + 1) * P],
                            rhs=at8_sb[:, cc, :, :],
                            start=(cc == 0), stop=(cc == CC - 1), perf_mode=DR,
                        )
                    nc.scalar.activation(
                        out=aggF[fch // 2][:, fch % 2, :], in_=pag[:],
                        func=Ident, bias=0.0, scale=float(2.0 ** (AG - AM)),
                    )

                # prefetch next layer's weights while gates run
                if l + 1 < L:
                    wl_next = wlp.tile([P, KK, 2, D], fp8, tag="wlp",
                                       name=f"wl{l + 1}")
                    nc.scalar.dma_start(out=wl_next[:], in_=wl8[l + 1])
                    wcn_next = [
                        wcnp.tile([P, D], f32r, tag="wcnp",
                                  name=f"wcn{l + 1}_{k}")
                        for k in range(DCH)
                    ]
                    for k in range(DCH):
                        eng = nc.sync if k % 2 else nc.scalar
                        eng.dma_start(out=wcn_next[k][:], in_=wcn[l + 1, k])

                # GRU gates, 128 gate rows at a time (feature chunk i)
                hTn = []
                hFn = [
                    hf8.tile([P, 2, NF], fp8, tag="hf8",
                             name=f"h{l + 1}q{kk}")
                    for kk in range(KK)
                ] if l + 1 < L else []
                # stage-skewed gate pipeline: A(i) = matmuls + sigmoids,
                # B(i) = rn/tn/tanh, C(i) = h' update + fp8 shadow.  Skewing
                # keeps every engine's in-order queue dependency-ready.
                st = [dict() for _ in range(DCH)]

                def stageA(i):
                    rz = []
                    for gi_, (g, pg) in enumerate(
                        ((i, rz_ps[i][0]), (DCH + i, rz_ps[i][1]))
                    ):
                        for kk in range(KK):
                            nc.tensor.matmul(
                                out=pg[:], lhsT=wih_sb[:, g, kk, :, :],
                                rhs=aggF[kk][:],
                                start=False, stop=(kk == KK - 1), perf_mode=DR,
                            )
                        gs = gp.tile([P, NF], f32, tag="gp",
                                     name=f"g{l}_{i}_{gi_}")
                        nc.scalar.activation(
                            out=gs[:], in_=pg[:], func=Sigmoid,
                            bias=bsum_sb[:, g : g + 1],
                            scale=float(2.0 ** -SG),
                        )
                        rz.append(gs)
                    del rz_ps[i]
                    if i + 1 < DCH and i + 1 not in rz_ps:
                        emit_gh_rz(i + 1)
                    st[i]["r"], st[i]["z"] = rz

                def stageB(i):
                    s_i = st[i]
                    g = 2 * DCH + i
                    pghn = psN.tile([P, NF], f32, tag="psN",
                                    name=f"pghn{l}_{i}")
                    for kk in range(KK):
                        nc.tensor.matmul(
                            out=pghn[:], lhsT=whh_sb[:, g, kk, :, :],
                            rhs=hF[kk][:],
                            start=(kk == 0), stop=(kk == KK - 1), perf_mode=DR,
                        )
                    pgin = psN.tile([P, NF], f32, tag="psN",
                                    name=f"pgin{l}_{i}")
                    for c in range(NCH):
                        nc.tensor.matmul(
                            out=pgin[:],
                            lhsT=mpN[c][:, i * P : (i + 1) * P],
                            rhs=atf_sb[:, c, :],
                            start=(c == 0), stop=(c == NCH - 1),
                        )
                    s_i["pghn"], s_i["pgin"] = pghn, pgin
                    rn = gp.tile([P, NF], f32, tag="gp", name=f"rn{l}_{i}")
                    if has_bhhn:
                        hb = gp.tile([P, NF], f32, tag="gp",
                                     name=f"hb{l}_{i}")
                        nc.scalar.activation(
                            out=hb[:], in_=s_i["pghn"][:], func=Ident,
                            bias=bhhn_sb[:, i : i + 1], scale=1.0,
                        )
                        nc.vector.tensor_mul(out=rn[:], in0=s_i["r"][:],
                                             in1=hb[:])
                    else:
                        nc.vector.tensor_mul(out=rn[:], in0=s_i["r"][:],
                                             in1=s_i["pghn"][:])
                    tn = gp.tile([P, NF], f32, tag="gp", name=f"tn{l}_{i}")
                    nc.vector.tensor_add(out=tn[:], in0=s_i["pgin"][:],
                                         in1=rn[:])
                    nn_ = gp.tile([P, NF], f32, tag="gp", name=f"nn{l}_{i}")
                    nc.scalar.activation(
                        out=nn_[:], in_=tn[:], func=Tanh,
                        bias=bihn_sb[:, i : i + 1], scale=float(2.0 ** -SG),
                    )
                    s_i["n"] = nn_

                def stageC(i):
                    s_i = st[i]
                    nn_ = s_i["n"]
                    s_ = gp.tile([P, NF], f32, tag="gp", name=f"s{l}_{i}")
                    nc.gpsimd.tensor_sub(out=s_[:], in0=hT[i][:], in1=nn_[:])
                    sz = gp.tile([P, NF], f32, tag="gp", name=f"sz{l}_{i}")
                    nc.gpsimd.tensor_mul(out=sz[:], in0=s_i["z"][:], in1=s_[:])
                    hj = hp.tile([P, NF], f32r, tag="hp",
                                 name=f"h{l + 1}_{i}")
                    nc.vector.tensor_add(out=hj[:], in0=nn_[:], in1=sz[:])
                    hTn.append(hj)
                    if l + 1 < L:
                        nc.vector.tensor_scalar_mul(
                            out=hFn[i // 2][:, i % 2, :], in0=hj[:],
                            scalar1=float(2 ** AH),
                        )

                for t in range(DCH + 2):
                    if t < DCH:
                        stageA(t)
                    if 1 <= t + 0 and 0 <= t - 1 < DCH:
                        stageB(t - 1)
                    if 0 <= t - 2 < DCH:
                        stageC(t - 2)
                hT, hF = hTn, hFn
                if l + 1 < L:
                    wl_sb = wl_next
                    wcn_sb = wcn_next

            # ---- transpose back to node-major, mask, write out ----
            # j-major: transposes for feature chunk j start as soon as the
            # last layer's h'(j) lands, overlapping the pointwise drain
            poas = [ps5.tile([P, NF], f32, tag="ps5", name=f"poa{i}")
                    for i in range(NCH)]
            pobs = [psN.tile([P, D - NF], f32, tag="psN", name=f"pob{i}")
                    for i in range(NCH)]
            for j in range(DCH):
                for i in range(NCH):
                    dst = poas[i][:, j * P : (j + 1) * P] if j < 4 else \
                        pobs[i][:, (j - 4) * P : (j - 3) * P]
                    nc.tensor.transpose(
                        out=dst,
                        in_=hT[j][:, i * P : (i + 1) * P].bitcast(f32),
                        identity=identity[:],
                    )
            for i in range(NCH):
                ob = obp.tile([P, D], f32, tag="obp")
                nc.vector.tensor_scalar_mul(
                    out=ob[:, :NF], in0=poas[i][:],
                    scalar1=mask_sb[:, i : i + 1],
                )
                nc.scalar.activation(
                    out=ob[:, NF:D], in_=pobs[i][:], func=Ident, bias=0.0,
                    scale=mask_sb[:, i : i + 1],
                )
                eng = nc.sync if i % 2 else nc.scalar
                eng.dma_start(out=out[i * P : (i + 1) * P, :NF],
                              in_=ob[:, :NF])
                eng2 = nc.scalar if i % 2 else nc.sync
                eng2.dma_start(out=out[i * P : (i + 1) * P, NF:D],
                               in_=ob[:, NF:D])

    nc.compile()
    return nc


@functools.lru_cache(maxsize=4)
def _get_nc(pool_wide: bool, has_bhhn: bool) -> bass.Bass:
    return build_nc(pool_wide, has_bhhn)


def _prep_shared(inputs, pool_wide):
    """Weight tensors identical across graphs, pre-quantized / pre-laid-out
    partition-major so every DMA is contiguous per partition."""
    word = np.asarray(inputs["word_emb"], np.float32)
    tt = np.ascontiguousarray(np.asarray(inputs["type_table"], np.float32))
    fw = np.asarray(inputs["fusion_w"], np.float32)          # [F, D]
    # [p, j, c, m] = fw[c*128+p, j*128+m]
    fusion_w = np.ascontiguousarray(
        fw.reshape(FCH, P, DCH, P).transpose(1, 2, 0, 3)
    ).astype(NPBF)
    wl_w = np.asarray(inputs["ggnn_w"], np.float32)          # [L, D, D]
    wl8 = np.ascontiguousarray(
        (wl_w * 2.0 ** BWL).reshape(L, KK, 2, P, D).transpose(0, 3, 1, 2, 4)
    ).astype(NPF8)
    wih_w = np.asarray(inputs["gru_w_ih"], np.float32)       # [3D, D]
    whh_w = np.asarray(inputs["gru_w_hh"], np.float32)
    # [p, g, kk, i, m] = W.T[(2kk+i)*128+p, g*128+m] * scale
    wih8 = np.ascontiguousarray(
        (wih_w[: 2 * D].T * 2.0 ** BIH)
        .reshape(KK, 2, P, 2 * DCH, P).transpose(2, 3, 0, 1, 4)
    ).astype(NPF8)
    whh8 = np.ascontiguousarray(
        (whh_w.T * 2.0 ** BHH)
        .reshape(KK, 2, P, GCH, P).transpose(2, 3, 0, 1, 4)
    ).astype(NPF8)
    # Wcn_l = Wl @ Wih_n^T, pre-scaled by 2^SG; [l, k, p, j]
    wih_n = wih_w[2 * D :]
    wcn = np.stack([
        (wl_w[l].astype(np.float64) @ wih_n.T.astype(np.float64))
        for l in range(L)
    ]).astype(np.float32) * 2.0 ** SG
    wcn = np.ascontiguousarray(wcn.reshape(L, DCH, P, D))
    bih = np.asarray(inputs["gru_b_ih"], np.float32)
    bhh = np.asarray(inputs["gru_b_hh"], np.float32)
    smalls = np.zeros((P, 34), np.float32)
    smalls[:, 0:12] = (bih[: 2 * D] + bhh[: 2 * D]).reshape(2 * DCH, P).T
    smalls[:, 12:18] = bih[2 * D :].reshape(DCH, P).T
    smalls[:, 18:24] = (bhh[2 * D :] * 2.0 ** SG).reshape(DCH, P).T
    smalls[:, 24:30] = (
        np.asarray(inputs["fusion_b"], np.float32).reshape(DCH, P).T
    )
    shared = dict(
        type_table=tt, fusion_w=fusion_w, wl8=wl8, wcn=wcn, wih8=wih8,
        whh8=whh8, _smalls_base=smalls,
    )
    shared["word_bf"] = np.ascontiguousarray(word.astype(NPBF))
    return shared


def _graph_blockable(inputs, b):
    seg = np.asarray(inputs["token_seg_ids"][b], np.int64)
    tcol = np.arange(T) // P
    return bool(np.all((seg >= tcol * BLK) & (seg < (tcol + 1) * BLK)))


def _prep_graph(inputs, b, pool_wide):
    tok = np.asarray(inputs["node_token_ids"][b], np.int64)
    typ = np.asarray(inputs["node_types"][b], np.int32)
    seg = np.asarray(inputs["token_seg_ids"][b], np.int64)
    lens = np.asarray(inputs["node_token_lens"][b], np.float64)
    glen = int(np.asarray(inputs["graph_node_lens"][b]))
    esrc = np.asarray(inputs["edge_src"][b], np.int64)
    edst = np.asarray(inputs["edge_dst"][b], np.int64)
    ew = np.asarray(inputs["edge_weight"][b], np.float32)

    # token idxs for dma_gather: GS splits of GT idxs, each wrapped into
    # 16 partitions and replicated to 128 partitions
    tok16 = tok.astype(np.int16)
    cols = []
    for s in range(GS):
        w16 = tok16[s * GT : (s + 1) * GT].reshape(GT // 16, 16).T
        cols.append(np.tile(w16, (8, 1)))
    tok_idx = np.ascontiguousarray(np.concatenate(cols, axis=1))

    typ_oh = np.zeros((TYPES, N), np.float32)
    typ_oh[typ, np.arange(N)] = 1.0

    # dense adjacency A[src, dst]: f32r copy + DoubleRow-paired fp8 copy
    at = np.zeros((N, N), np.float32)
    np.add.at(at, (esrc, edst), ew)
    at_f = np.ascontiguousarray(
        at.reshape(NCH, P, N).transpose(1, 0, 2)
    ).astype(NPBF)
    at_8 = np.ascontiguousarray(
        at.reshape(CC, 2, P, N).transpose(2, 0, 1, 3)
    ).astype(NPF8)

    winv = np.zeros(N, np.float64)
    nzmask = lens != 0
    winv[nzmask] = 1.0 / lens[nzmask]
    t_ = np.arange(T)
    c_ = t_ // P
    if pool_wide:
        pm = np.zeros((TCH, P, N), np.float32)
        pm[c_, t_ % P, seg] = winv[seg]
        pool_arr = np.ascontiguousarray(pm.transpose(1, 0, 2)).astype(NPBF)
        pool_key = "poolw"
    else:
        pm = np.zeros((TCH, P, BLK), np.float32)
        pm[c_, t_ % P, seg - c_ * BLK] = winv[seg]
        pool_arr = np.ascontiguousarray(pm.transpose(1, 0, 2)).astype(NPBF)
        pool_key = "poolm"

    keep = min(glen, MAX_NODE_LEN)
    mask = np.ascontiguousarray(
        (np.arange(N) < keep).astype(np.float32).reshape(NCH, P).T
    )
    return {
        "tok_idx": tok_idx, "typ_oh": typ_oh, "at_f": at_f, "at_8": at_8,
        pool_key: pool_arr, "_mask": mask,
    }


def kernel(**inputs) -> np.ndarray:
    pool_wide = not all(_graph_blockable(inputs, b) for b in range(B))
    has_bhhn = bool(
        np.any(np.asarray(inputs["gru_b_hh"], np.float32)[2 * D :] != 0.0)
    )
    shared = _prep_shared(inputs, pool_wide)
    smalls_base = shared.pop("_smalls_base")
    per_graph = [_prep_graph(inputs, b, pool_wide) for b in range(B)]
    in_maps = []
    for b in range(B):
        g = dict(per_graph[b])
        sm = smalls_base.copy()
        sm[:, 30:34] = g.pop("_mask")
        g["smalls"] = sm
        in_maps.append({**shared, **g})
    nc = _get_nc(pool_wide, has_bhhn)
    res = bass_utils.run_bass_kernel_spmd(nc, in_maps, core_ids=list(range(B)))
    global _last_exec_ns
    _last_exec_ns = res.exec_time_ns
    out = np.stack([r["out"] for r in res.results]).astype(np.float32)
    return out


_last_exec_ns = None
_last_nc = None


# revision 5
# speedup vs baseline: 1.7707x; 1.0296x over previous
# GGNN encoder kernel for Trainium2 (Bass/Tile), data-parallel over the
# batch dimension: 8 graphs -> 8 NeuronCores, one graph per core.
#
# Mixed-precision design ("R2"):
#  - r/z gates and gh_n run as fp8(e4m3) DoubleRow matmuls (two 128-deep
#    K-planes per instruction at 0.5 cycles/row).  Their quantization
#    noise is squashed by the sigmoid (and by r*gh_n being small).
#  - The sensitive n-gate input gi_n = A^T (h @ Wl @ Wih_n^T) is computed
#    exactly in f32r via a host-side weight fold (Wcn = Wl @ Wih_n^T),
#    which also deletes the separate per-layer m matmul from this path.
#  - h master stays fp32; embeddings/pooling/fusion run in bf16/f32r.
#  - All fp8 operands carry power-of-two scales, folded exactly into the
#    activation-function scale arguments.
#
# Per-core computation (one graph):
#   type_e  = type_table[node_types]                       # f32r one-hot matmul
#   tok_e   = word_emb[node_token_ids]                     # bf16 SWDGE dma_gather
#   text_e  = segment_mean(tok_e, token_seg_ids)           # bf16 pooling matmul
#   h       = concat(type_e, text_e) @ fusion_w + b        # bf16 matmul
#   4 x GGNN layer:
#     m8   = h8 @ Wl8          (fp8 DR)      -> agg8 = A8-contract (fp8 DR)
#     r,z  = sigmoid(Wih8 agg8 + Whh8 h8 + b)              (fp8 DR psum)
#     gin  = A-contract(h @ Wcn)                           (f32r, exact)
#     ghn  = Whh_n8 h8                                     (fp8 DR)
#     n    = tanh(gin + r*ghn + b);  h' = n + z*(h - n)    (fp32 pointwise)
#   out     = mask * h                                     # PE transpose epilogue

import functools

import numpy as np
import ml_dtypes

import concourse.bass as bass
import concourse.mybir as mybir
import concourse.tile as tile
from concourse import bacc, bass_utils
from concourse.masks import make_identity

# Problem shapes (hardcoded: kernel must be self-contained).
B, N, T, D, TD, L = 8, 512, 2048, 768, 128, 4
V, TYPES = 30522, 64
MAX_NODE_LEN = 512
F = TD + D            # 896 fused embedding dim
P = 128               # partitions
DCH = D // P          # 6 feature chunks
KK = DCH // 2         # 3 feature chunk pairs (DoubleRow planes)
NCH = N // P          # 4 node chunks
CC = NCH // 2         # 2 node chunk pairs
FCH = F // P          # 7 fused chunks
GCH = 3 * DCH         # 18 gate row chunks
TCH = T // P          # 16 token chunks
BLK = N // TCH        # 32 nodes per token chunk (block-pooling case)
NF = 512              # free-dim tile (nodes)
GS = 4                # token gather splits
GT = T // GS          # tokens per gather split (512)
GC = GT // P          # 128-chunks per gather split (4)

# power-of-two scale exponents for the fp8 operands
AH = 4                # h -> fp8
AM = 9                # m -> fp8
AG = 4                # agg -> fp8
BWL = 5               # ggnn_w
BIH = 5               # gru_w_ih (r,z rows)
BHH = 5               # gru_w_hh
SG = BIH + AG         # gate psum scale (== BHH + AH); Wcn also pre-scaled 2^SG
assert SG == BHH + AH

f32 = mybir.dt.float32
f32r = mybir.dt.float32r
bf16 = mybir.dt.bfloat16
fp8 = mybir.dt.float8e4
i16 = mybir.dt.int16
NPF8 = ml_dtypes.float8_e4m3
NPBF = ml_dtypes.bfloat16

Sigmoid = mybir.ActivationFunctionType.Sigmoid
Tanh = mybir.ActivationFunctionType.Tanh
Ident = mybir.ActivationFunctionType.Identity
DR = mybir.MatmulPerfMode.DoubleRow


def build_nc(pool_wide: bool, has_bhhn: bool) -> bass.Bass:
    nc = bacc.Bacc(num_swdge_queues=2, dynamic_dma_scratch_size=32768)

    tok_idx = nc.dram_tensor("tok_idx", [P, T // 16], i16, kind="ExternalInput")
    typ_oh = nc.dram_tensor("typ_oh", [TYPES, N], f32r, kind="ExternalInput")
    type_table = nc.dram_tensor("type_table", [TYPES, TD], f32r,
                                kind="ExternalInput")
    word_d = nc.dram_tensor("word_bf", [V, D], bf16, kind="ExternalInput")
    if pool_wide:
        poolm = nc.dram_tensor("poolw", [P, TCH, N], bf16, kind="ExternalInput")
    else:
        poolm = nc.dram_tensor("poolm", [P, TCH, BLK], bf16,
                               kind="ExternalInput")
    # fusion_w: [p, j, c, m] (bf16, one DMA)
    fusion_w = nc.dram_tensor("fusion_w", [P, DCH, FCH, P], bf16,
                              kind="ExternalInput")
    at_f = nc.dram_tensor("at_f", [P, NCH, N], bf16, kind="ExternalInput")
    at_8 = nc.dram_tensor("at_8", [P, CC, 2, N], fp8, kind="ExternalInput")
    wl8 = nc.dram_tensor("wl8", [L, P, KK, 2, D], fp8, kind="ExternalInput")
    wcn = nc.dram_tensor("wcn", [L, DCH, P, D], f32r, kind="ExternalInput")
    wih8 = nc.dram_tensor("wih8", [P, 2 * DCH, KK, 2, P], fp8,
                          kind="ExternalInput")
    whh8 = nc.dram_tensor("whh8", [P, GCH, KK, 2, P], fp8,
                          kind="ExternalInput")
    # smalls: [bsum(12) | bihn(6) | bhhn_s(6) | fusion_b(6) | mask(4)]
    smalls = nc.dram_tensor("smalls", [P, 34], f32, kind="ExternalInput")
    out = nc.dram_tensor("out", [N, D], f32, kind="ExternalOutput")

    wide = pool_wide
    with tile.TileContext(nc) as tc:
        with (
            tc.tile_pool(name="consts", bufs=1) as consts,
            tc.tile_pool(name="wts", bufs=1) as wts,
            tc.tile_pool(name="wlp", bufs=2) as wlp,
            tc.tile_pool(name="wcnp", bufs=6) as wcnp,
            tc.tile_pool(name="tokg", bufs=4) as tokg,
            tc.tile_pool(name="hp", bufs=12) as hp,
            tc.tile_pool(name="hf8", bufs=5 if wide else 6) as hf8,
            tc.tile_pool(name="mnp", bufs=4) as mnp,
            tc.tile_pool(name="mfp", bufs=5) as mfp,
            tc.tile_pool(name="aggp", bufs=4 if wide else 5) as aggp,
            tc.tile_pool(name="gp", bufs=11 if wide else 17) as gp,
            tc.tile_pool(name="obp", bufs=2) as obp,
            tc.tile_pool(name="ps5", bufs=6, space="PSUM") as ps5,
            tc.tile_pool(name="psN", bufs=2, space="PSUM") as psN,
        ):
            # ---- token gather first: it gates the whole front of the kernel
            tok_idx_sb = consts.tile([P, T // 16], i16)
            nc.sync.dma_start(out=tok_idx_sb[:], in_=tok_idx[:])
            if wide:
                pool_sb = consts.tile([P, TCH, N], bf16)
            else:
                pool_sb = consts.tile([P, TCH, BLK], bf16)
            nc.sync.dma_start(out=pool_sb[:], in_=poolm[:])

            tt_sb = consts.tile([TYPES, TD], f32r)
            nc.sync.dma_start(out=tt_sb[:], in_=type_table[:])
            oh_sb = consts.tile([TYPES, N], f32r)
            nc.sync.dma_start(out=oh_sb[:], in_=typ_oh[:])

            # fusion weights ahead of the token gathers on the DMA pipe:
            # fusion is the first consumer after pooling
            fw_sb = wts.tile([P, DCH, FCH, P], bf16)
            nc.scalar.dma_start(out=fw_sb[:], in_=fusion_w[:])
            wl_sb = wlp.tile([P, KK, 2, D], fp8, tag="wlp", name="wl0")
            nc.scalar.dma_start(out=wl_sb[:], in_=wl8[0])
            at8_sb = wts.tile([P, CC, 2, N], fp8)
            nc.scalar.dma_start(out=at8_sb[:], in_=at_8[:])

            gdt = bf16
            gath = []
            for s in range(GS):
                tg = tokg.tile([P, GC, D], gdt, tag="tokg", name=f"tokg{s}")
                nc.gpsimd.dma_gather(
                    tg[:], word_d[:],
                    tok_idx_sb[:, s * (GT // 16) : (s + 1) * (GT // 16)],
                    GT, GT, D, queue_num=s % 2,
                )
                gath.append(tg)

            # ---- remaining constants / small inputs ----
            identity = consts.tile([P, P], f32)
            make_identity(nc, identity[:])
            smalls_sb = consts.tile([P, 34], f32)
            nc.sync.dma_start(out=smalls_sb[:], in_=smalls[:])
            bsum_sb = smalls_sb[:, 0:12]
            bihn_sb = smalls_sb[:, 12:18]
            bhhn_sb = smalls_sb[:, 18:24]
            fb_sb = smalls_sb[:, 24:30]
            mask_sb = smalls_sb[:, 30:34]

            # remaining weights in the order the DMA pipe must serve them
            whh_sb = wts.tile([P, GCH, KK, 2, P], fp8)
            nc.sync.dma_start(out=whh_sb[:], in_=whh8[:])
            wih_sb = wts.tile([P, 2 * DCH, KK, 2, P], fp8)
            nc.scalar.dma_start(out=wih_sb[:], in_=wih8[:])
            wcn_sb = [
                wcnp.tile([P, D], f32r, tag="wcnp", name=f"wcn0_{k}")
                for k in range(DCH)
            ]
            for k in range(DCH):
                eng = nc.sync if k % 2 else nc.scalar
                eng.dma_start(out=wcn_sb[k][:], in_=wcn[0, k])
            atf_sb = wts.tile([P, NCH, N], bf16)
            nc.sync.dma_start(out=atf_sb[:], in_=at_f[:])

            # fused embedding, feature-major bf16: chunk 0 = type_e,
            # chunks 1..6 = text_e
            fusedT = [
                consts.tile([P, NF], bf16, name=f"fusedT{k}")
                for k in range(FCH)
            ]

            # type_eT = type_table.T @ onehot  (f32r, K=64)
            ptyp = ps5.tile([P, NF], f32, tag="ps5")
            nc.tensor.matmul(
                out=ptyp[:], lhsT=tt_sb[:], rhs=oh_sb[:], start=True, stop=True
            )
            nc.scalar.activation(out=fusedT[0][:], in_=ptyp[:], func=Ident,
                                 bias=0.0)

            # token pooling: 128 tokens -> 32 nodes (block) / 512 (wide)
            if wide:
                # c-major accumulation: each gather split is consumed once
                # and released (tokg ring holds only 2 splits in wide mode)
                pfs = [
                    ps5.tile([P, NF], f32, tag="ps5", name=f"pf{fch}")
                    for fch in range(DCH)
                ]
                for c in range(TCH):
                    s, c2 = divmod(c, GC)
                    for fch in range(DCH):
                        nc.tensor.matmul(
                            out=pfs[fch][:],
                            lhsT=gath[s][:, c2, fch * P : (fch + 1) * P],
                            rhs=pool_sb[:, c, :],
                            start=(c == 0), stop=(c == TCH - 1),
                        )
                for fch in range(DCH):
                    nc.scalar.activation(out=fusedT[1 + fch][:],
                                         in_=pfs[fch][:], func=Ident,
                                         bias=0.0)
            else:
                for fch in range(DCH):
                    pf = ps5.tile([P, NF], f32, tag="ps5", name=f"pf{fch}")
                    for c in range(TCH):
                        s, c2 = divmod(c, GC)
                        nc.tensor.matmul(
                            out=pf[:, c * BLK : (c + 1) * BLK],
                            lhsT=gath[s][:, c2, fch * P : (fch + 1) * P],
                            rhs=pool_sb[:, c, :],
                            start=True, stop=True,
                        )
                    nc.scalar.activation(out=fusedT[1 + fch][:], in_=pf[:],
                                         func=Ident, bias=0.0)

            # ---- fusion matmul: h0 feature-major f32 + fp8 shadow ----
            hT = []
            hF = [
                hf8.tile([P, 2, NF], fp8, tag="hf8", name=f"h0q{kk}")
                for kk in range(KK)
            ]
            for j in range(DCH):
                pfj = ps5.tile([P, NF], f32, tag="ps5")
                for c in range(FCH):
                    nc.tensor.matmul(
                        out=pfj[:],
                        lhsT=fw_sb[:, j, c, :],
                        rhs=fusedT[c][:],
                        start=(c == 0), stop=(c == FCH - 1),
                    )
                hj = hp.tile([P, NF], f32r, tag="hp", name=f"h0_{j}")
                nc.scalar.activation(
                    out=hj[:], in_=pfj[:], func=Ident,
                    bias=fb_sb[:, j : j + 1],
                )
                hT.append(hj)
                nc.vector.tensor_scalar_mul(
                    out=hF[j // 2][:, j % 2, :], in0=hj[:],
                    scalar1=float(2 ** AH),
                )

            # ---- GGNN layers ----
            for l in range(L):
                # m8 = h8 @ Wl8 (fp8 DR) and m' = h @ Wcn (f32r, exact),
                # both node-major, chunk-interleaved so the f32r m' work
                # covers the psum->sbuf copy latency of m8
                mN = [
                    mnp.tile([P, 2, D], fp8, tag="mnp", name=f"mN{l}_{cc}")
                    for cc in range(CC)
                ]
                mpN = [
                    mfp.tile([P, D], bf16, tag="mfp", name=f"mp{l}_{i}")
                    for i in range(NCH)
                ]

                def emit_m8(i):
                    pma = ps5.tile([P, NF], f32, tag="ps5", name=f"pma{l}_{i}")
                    pmb = ps5.tile([P, D - NF], f32, tag="ps5",
                                   name=f"pmb{l}_{i}")
                    for kk in range(KK):
                        lh = hF[kk][:, :, i * P : (i + 1) * P]
                        nc.tensor.matmul(
                            out=pma[:], lhsT=lh, rhs=wl_sb[:, kk, :, :NF],
                            start=(kk == 0), stop=(kk == KK - 1), perf_mode=DR,
                        )
                        nc.tensor.matmul(
                            out=pmb[:], lhsT=lh, rhs=wl_sb[:, kk, :, NF:D],
                            start=(kk == 0), stop=(kk == KK - 1), perf_mode=DR,
                        )
                    nc.scalar.activation(
                        out=mN[i // 2][:, i % 2, :NF], in_=pma[:],
                        func=Ident, bias=0.0,
                    )
                    nc.vector.tensor_copy(
                        out=mN[i // 2][:, i % 2, NF:D], in_=pmb[:]
                    )

                def emit_mp(i):
                    pca = ps5.tile([P, NF], f32, tag="ps5", name=f"pca{l}_{i}")
                    pcb = ps5.tile([P, D - NF], f32, tag="ps5",
                                   name=f"pcb{l}_{i}")
                    for k in range(DCH):
                        lh = hT[k][:, i * P : (i + 1) * P]
                        nc.tensor.matmul(
                            out=pca[:], lhsT=lh, rhs=wcn_sb[k][:, :NF],
                            start=(k == 0), stop=(k == DCH - 1),
                        )
                        nc.tensor.matmul(
                            out=pcb[:], lhsT=lh, rhs=wcn_sb[k][:, NF:D],
                            start=(k == 0), stop=(k == DCH - 1),
                        )
                    nc.scalar.activation(out=mpN[i][:, :NF], in_=pca[:],
                                         func=Ident, bias=0.0)
                    nc.vector.tensor_copy(out=mpN[i][:, NF:D], in_=pcb[:])

                # m' first: it reads the f32 master h directly (no
                # dependence on the fp8 quantize tail of the previous layer);
                # contract k=0..3 first so the PE has work before the last
                # h chunks of the previous layer finish
                mp_ps = {}
                for i in range(NCH):
                    pca = ps5.tile([P, NF], f32, tag="ps5",
                                   name=f"pca{l}_{i}")
                    pcb = ps5.tile([P, D - NF], f32, tag="ps5",
                                   name=f"pcb{l}_{i}")
                    mp_ps[i] = (pca, pcb)
                    for k in range(4):
                        lh = hT[k][:, i * P : (i + 1) * P]
                        nc.tensor.matmul(
                            out=pca[:], lhsT=lh, rhs=wcn_sb[k][:, :NF],
                            start=(k == 0), stop=False,
                        )
                        nc.tensor.matmul(
                            out=pcb[:], lhsT=lh, rhs=wcn_sb[k][:, NF:D],
                            start=(k == 0), stop=False,
                        )
                    if i % 2 == 1:
                        for j in (i - 1, i):
                            pca_, pcb_ = mp_ps[j]
                            for k in range(4, DCH):
                                lh = hT[k][:, j * P : (j + 1) * P]
                                nc.tensor.matmul(
                                    out=pca_[:], lhsT=lh,
                                    rhs=wcn_sb[k][:, :NF],
                                    start=False, stop=(k == DCH - 1),
                                )
                                nc.tensor.matmul(
                                    out=pcb_[:], lhsT=lh,
                                    rhs=wcn_sb[k][:, NF:D],
                                    start=False, stop=(k == DCH - 1),
                                )
                            nc.scalar.activation(out=mpN[j][:, :NF],
                                                 in_=pca_[:], func=Ident,
                                                 bias=0.0)
                            nc.vector.tensor_copy(out=mpN[j][:, NF:D],
                                                  in_=pcb_[:])
                for i in range(NCH):
                    emit_m8(i)

                # pre-open r/z psum groups with their gh contributions so
                # the PE has work while agg8 quantizes drain
                rz_ps = {}

                def emit_gh_rz(i):
                    prs = []
                    for g in (i, DCH + i):
                        pg = ps5.tile([P, NF], f32, tag="ps5",
                                      name=f"prz{l}_{g}")
                        for kk in range(KK):
                            nc.tensor.matmul(
                                out=pg[:], lhsT=whh_sb[:, g, kk, :, :],
                                rhs=hF[kk][:],
                                start=(kk == 0), stop=False, perf_mode=DR,
                            )
                        prs.append(pg)
                    rz_ps[i] = prs

                emit_gh_rz(0)

                # agg8 = A8-contraction of m8 (fp8 DR), feature-major
                aggF = [
                    aggp.tile([P, 2, NF], fp8, tag="aggp", name=f"agg{l}_{kk}")
                    for kk in range(KK)
                ]
                for fch in range(DCH):
                    pag = ps5.tile([P, NF], f32, tag="ps5")
                    for cc in range(CC):
                        nc.tensor.matmul(
                            out=pag[:],
                            lhsT=mN[cc][:, :, fch * P : (fch + 1) * P],
                            rhs=at8_sb[:, cc, :, :],
                            start=(cc == 0), stop=(cc == CC - 1), perf_mode=DR,
                        )
                    if fch % 2 == 0:
                        nc.scalar.activation(
                            out=aggF[fch // 2][:, fch % 2, :], in_=pag[:],
                            func=Ident, bias=0.0,
                            scale=float(2.0 ** (AG - AM)),
                        )
                    else:
                        nc.vector.tensor_scalar_mul(
                            out=aggF[fch // 2][:, fch % 2, :], in0=pag[:],
                            scalar1=float(2.0 ** (AG - AM)),
                        )

                # prefetch next layer's weights while gates run
                if l + 1 < L:
                    wl_next = wlp.tile([P, KK, 2, D], fp8, tag="wlp",
                                       name=f"wl{l + 1}")
                    nc.scalar.dma_start(out=wl_next[:], in_=wl8[l + 1])
                    wcn_next = [
                        wcnp.tile([P, D], f32r, tag="wcnp",
                                  name=f"wcn{l + 1}_{k}")
                        for k in range(DCH)
                    ]
                    for k in range(DCH):
                        eng = nc.sync if k % 2 else nc.scalar
                        eng.dma_start(out=wcn_next[k][:], in_=wcn[l + 1, k])

                # GRU gates, 128 gate rows at a time (feature chunk i)
                hTn = []
                hFn = [
                    hf8.tile([P, 2, NF], fp8, tag="hf8",
                             name=f"h{l + 1}q{kk}")
                    for kk in range(KK)
                ] if l + 1 < L else []
                # stage-skewed gate pipeline: A(i) = matmuls + sigmoids,
                # B(i) = rn/tn/tanh, C(i) = h' update + fp8 shadow.  Skewing
                # keeps every engine's in-order queue dependency-ready.
                st = [dict() for _ in range(DCH)]

                def stageA(i):
                    rz = []
                    for gi_, (g, pg) in enumerate(
                        ((i, rz_ps[i][0]), (DCH + i, rz_ps[i][1]))
                    ):
                        for kk in range(KK):
                            nc.tensor.matmul(
                                out=pg[:], lhsT=wih_sb[:, g, kk, :, :],
                                rhs=aggF[kk][:],
                                start=False, stop=(kk == KK - 1), perf_mode=DR,
                            )
                        gs = gp.tile([P, NF], f32, tag="gp",
                                     name=f"g{l}_{i}_{gi_}")
                        nc.scalar.activation(
                            out=gs[:], in_=pg[:], func=Sigmoid,
                            bias=bsum_sb[:, g : g + 1],
                            scale=float(2.0 ** -SG),
                        )
                        rz.append(gs)
                    del rz_ps[i]
                    if i + 1 < DCH and i + 1 not in rz_ps:
                        emit_gh_rz(i + 1)
                    st[i]["r"], st[i]["z"] = rz

                def stageB(i):
                    s_i = st[i]
                    g = 2 * DCH + i
                    pghn = psN.tile([P, NF], f32, tag="psN",
                                    name=f"pghn{l}_{i}")
                    for kk in range(KK):
                        nc.tensor.matmul(
                            out=pghn[:], lhsT=whh_sb[:, g, kk, :, :],
                            rhs=hF[kk][:],
                            start=(kk == 0), stop=(kk == KK - 1), perf_mode=DR,
                        )
                    pgin = psN.tile([P, NF], f32, tag="psN",
                                    name=f"pgin{l}_{i}")
                    for c in range(NCH):
                        nc.tensor.matmul(
                            out=pgin[:],
                            lhsT=mpN[c][:, i * P : (i + 1) * P],
                            rhs=atf_sb[:, c, :],
                            start=(c == 0), stop=(c == NCH - 1),
                        )
                    s_i["pghn"], s_i["pgin"] = pghn, pgin
                    rn = gp.tile([P, NF], f32, tag="gp", name=f"rn{l}_{i}")
                    if has_bhhn:
                        hb = gp.tile([P, NF], f32, tag="gp",
                                     name=f"hb{l}_{i}")
                        nc.scalar.activation(
                            out=hb[:], in_=s_i["pghn"][:], func=Ident,
                            bias=bhhn_sb[:, i : i + 1], scale=1.0,
                        )
                        nc.vector.tensor_mul(out=rn[:], in0=s_i["r"][:],
                                             in1=hb[:])
                    else:
                        nc.vector.tensor_mul(out=rn[:], in0=s_i["r"][:],
                                             in1=s_i["pghn"][:])
                    tn = gp.tile([P, NF], f32, tag="gp", name=f"tn{l}_{i}")
                    nc.vector.tensor_add(out=tn[:], in0=s_i["pgin"][:],
                                         in1=rn[:])
                    nn_ = gp.tile([P, NF], f32, tag="gp", name=f"nn{l}_{i}")
                    nc.scalar.activation(
                        out=nn_[:], in_=tn[:], func=Tanh,
                        bias=bihn_sb[:, i : i + 1], scale=float(2.0 ** -SG),
                    )
                    s_i["n"] = nn_

                def stageC(i):
                    s_i = st[i]
                    nn_ = s_i["n"]
                    s_ = gp.tile([P, NF], f32, tag="gp", name=f"s{l}_{i}")
                    nc.gpsimd.tensor_sub(out=s_[:], in0=hT[i][:], in1=nn_[:])
                    sz = gp.tile([P, NF], f32, tag="gp", name=f"sz{l}_{i}")
                    nc.gpsimd.tensor_mul(out=sz[:], in0=s_i["z"][:], in1=s_[:])
                    hj = hp.tile([P, NF], f32r, tag="hp",
                                 name=f"h{l + 1}_{i}")
                    nc.vector.tensor_add(out=hj[:], in0=nn_[:], in1=sz[:])
                    hTn.append(hj)
                    if l + 1 < L:
                        if i % 2 == 0:
                            nc.scalar.activation(
                                out=hFn[i // 2][:, i % 2, :], in_=hj[:],
                                func=Ident, bias=0.0, scale=float(2 ** AH),
                            )
                        else:
                            nc.vector.tensor_scalar_mul(
                                out=hFn[i // 2][:, i % 2, :], in0=hj[:],
                                scalar1=float(2 ** AH),
                            )

                for t in range(DCH + 2):
                    if t < DCH:
                        stageA(t)
                    if 1 <= t + 0 and 0 <= t - 1 < DCH:
                        stageB(t - 1)
                    if 0 <= t - 2 < DCH:
                        stageC(t - 2)
                hT, hF = hTn, hFn
                if l + 1 < L:
                    wl_sb = wl_next
                    wcn_sb = wcn_next

            # ---- transpose back to node-major, mask, write out ----
            # j-major: transposes for feature chunk j start as soon as the
            # last layer's h'(j) lands, overlapping the pointwise drain
            poas = [ps5.tile([P, NF], f32, tag="ps5", name=f"poa{i}")
                    for i in range(NCH)]
            pobs = [psN.tile([P, D - NF], f32, tag="psN", name=f"pob{i}")
                    for i in range(NCH)]
            for j in range(DCH):
                for i in range(NCH):
                    dst = poas[i][:, j * P : (j + 1) * P] if j < 4 else \
                        pobs[i][:, (j - 4) * P : (j - 3) * P]
                    nc.tensor.transpose(
                        out=dst,
                        in_=hT[j][:, i * P : (i + 1) * P].bitcast(f32),
                        identity=identity[:],
                    )
            for i in range(NCH):
                ob = obp.tile([P, D], f32, tag="obp")
                nc.vector.tensor_scalar_mul(
                    out=ob[:, :NF], in0=poas[i][:],
                    scalar1=mask_sb[:, i : i + 1],
                )
                nc.scalar.activation(
                    out=ob[:, NF:D], in_=pobs[i][:], func=Ident, bias=0.0,
                    scale=mask_sb[:, i : i + 1],
                )
                eng = nc.sync if i % 2 else nc.scalar
                eng.dma_start(out=out[i * P : (i + 1) * P, :NF],
                              in_=ob[:, :NF])
                eng2 = nc.scalar if i % 2 else nc.sync
                eng2.dma_start(out=out[i * P : (i + 1) * P, NF:D],
                               in_=ob[:, NF:D])

    nc.compile()
    return nc


@functools.lru_cache(maxsize=4)
def _get_nc(pool_wide: bool, has_bhhn: bool) -> bass.Bass:
    return build_nc(pool_wide, has_bhhn)


def _prep_shared(inputs, pool_wide):
    """Weight tensors identical across graphs, pre-quantized / pre-laid-out
    partition-major so every DMA is contiguous per partition."""
    word = np.asarray(inputs["word_emb"], np.float32)
    tt = np.ascontiguousarray(np.asarray(inputs["type_table"], np.float32))
    fw = np.asarray(inputs["fusion_w"], np.float32)          # [F, D]
    # [p, j, c, m] = fw[c*128+p, j*128+m]
    fusion_w = np.ascontiguousarray(
        fw.reshape(FCH, P, DCH, P).transpose(1, 2, 0, 3)
    ).astype(NPBF)
    wl_w = np.asarray(inputs["ggnn_w"], np.float32)          # [L, D, D]
    wl8 = np.ascontiguousarray(
        (wl_w * 2.0 ** BWL).reshape(L, KK, 2, P, D).transpose(0, 3, 1, 2, 4)
    ).astype(NPF8)
    wih_w = np.asarray(inputs["gru_w_ih"], np.float32)       # [3D, D]
    whh_w = np.asarray(inputs["gru_w_hh"], np.float32)
    # [p, g, kk, i, m] = W.T[(2kk+i)*128+p, g*128+m] * scale
    wih8 = np.ascontiguousarray(
        (wih_w[: 2 * D].T * 2.0 ** BIH)
        .reshape(KK, 2, P, 2 * DCH, P).transpose(2, 3, 0, 1, 4)
    ).astype(NPF8)
    whh8 = np.ascontiguousarray(
        (whh_w.T * 2.0 ** BHH)
        .reshape(KK, 2, P, GCH, P).transpose(2, 3, 0, 1, 4)
    ).astype(NPF8)
    # Wcn_l = Wl @ Wih_n^T, pre-scaled by 2^SG; [l, k, p, j]
    wih_n = wih_w[2 * D :]
    wcn = np.stack([
        (wl_w[l].astype(np.float64) @ wih_n.T.astype(np.float64))
        for l in range(L)
    ]).astype(np.float32) * 2.0 ** SG
    wcn = np.ascontiguousarray(wcn.reshape(L, DCH, P, D))
    bih = np.asarray(inputs["gru_b_ih"], np.float32)
    bhh = np.asarray(inputs["gru_b_hh"], np.float32)
    smalls = np.zeros((P, 34), np.float32)
    smalls[:, 0:12] = (bih[: 2 * D] + bhh[: 2 * D]).reshape(2 * DCH, P).T
    smalls[:, 12:18] = bih[2 * D :].reshape(DCH, P).T
    smalls[:, 18:24] = (bhh[2 * D :] * 2.0 ** SG).reshape(DCH, P).T
    smalls[:, 24:30] = (
        np.asarray(inputs["fusion_b"], np.float32).reshape(DCH, P).T
    )
    shared = dict(
        type_table=tt, fusion_w=fusion_w, wl8=wl8, wcn=wcn, wih8=wih8,
        whh8=whh8, _smalls_base=smalls,
    )
    shared["word_bf"] = np.ascontiguousarray(word.astype(NPBF))
    return shared


def _graph_blockable(inputs, b):
    seg = np.asarray(inputs["token_seg_ids"][b], np.int64)
    tcol = np.arange(T) // P
    return bool(np.all((seg >= tcol * BLK) & (seg < (tcol + 1) * BLK)))


def _prep_graph(inputs, b, pool_wide):
    tok = np.asarray(inputs["node_token_ids"][b], np.int64)
    typ = np.asarray(inputs["node_types"][b], np.int32)
    seg = np.asarray(inputs["token_seg_ids"][b], np.int64)
    lens = np.asarray(inputs["node_token_lens"][b], np.float64)
    glen = int(np.asarray(inputs["graph_node_lens"][b]))
    esrc = np.asarray(inputs["edge_src"][b], np.int64)
    edst = np.asarray(inputs["edge_dst"][b], np.int64)
    ew = np.asarray(inputs["edge_weight"][b], np.float32)

    # token idxs for dma_gather: GS splits of GT idxs, each wrapped into
    # 16 partitions and replicated to 128 partitions
    tok16 = tok.astype(np.int16)
    cols = []
    for s in range(GS):
        w16 = tok16[s * GT : (s + 1) * GT].reshape(GT // 16, 16).T
        cols.append(np.tile(w16, (8, 1)))
    tok_idx = np.ascontiguousarray(np.concatenate(cols, axis=1))

    typ_oh = np.zeros((TYPES, N), np.float32)
    typ_oh[typ, np.arange(N)] = 1.0

    # dense adjacency A[src, dst]: f32r copy + DoubleRow-paired fp8 copy
    at = np.zeros((N, N), np.float32)
    np.add.at(at, (esrc, edst), ew)
    at_f = np.ascontiguousarray(
        at.reshape(NCH, P, N).transpose(1, 0, 2)
    ).astype(NPBF)
    at_8 = np.ascontiguousarray(
        at.reshape(CC, 2, P, N).transpose(2, 0, 1, 3)
    ).astype(NPF8)

    winv = np.zeros(N, np.float64)
    nzmask = lens != 0
    winv[nzmask] = 1.0 / lens[nzmask]
    t_ = np.arange(T)
    c_ = t_ // P
    if pool_wide:
        pm = np.zeros((TCH, P, N), np.float32)
        pm[c_, t_ % P, seg] = winv[seg]
        pool_arr = np.ascontiguousarray(pm.transpose(1, 0, 2)).astype(NPBF)
        pool_key = "poolw"
    else:
        pm = np.zeros((TCH, P, BLK), np.float32)
        pm[c_, t_ % P, seg - c_ * BLK] = winv[seg]
        pool_arr = np.ascontiguousarray(pm.transpose(1, 0, 2)).astype(NPBF)
        pool_key = "poolm"

    keep = min(glen, MAX_NODE_LEN)
    mask = np.ascontiguousarray(
        (np.arange(N) < keep).astype(np.float32).reshape(NCH, P).T
    )
    return {
        "tok_idx": tok_idx, "typ_oh": typ_oh, "at_f": at_f, "at_8": at_8,
        pool_key: pool_arr, "_mask": mask,
    }


def kernel(**inputs) -> np.ndarray:
    pool_wide = not all(_graph_blockable(inputs, b) for b in range(B))
    has_bhhn = bool(
        np.any(np.asarray(inputs["gru_b_hh"], np.float32)[2 * D :] != 0.0)
    )
    shared = _prep_shared(inputs, pool_wide)
    smalls_base = shared.pop("_smalls_base")
    per_graph = [_prep_graph(inputs, b, pool_wide) for b in range(B)]
    in_maps = []
    for b in range(B):
        g = dict(per_graph[b])
        sm = smalls_base.copy()
        sm[:, 30:34] = g.pop("_mask")
        g["smalls"] = sm
        in_maps.append({**shared, **g})
    nc = _get_nc(pool_wide, has_bhhn)
    res = bass_utils.run_bass_kernel_spmd(nc, in_maps, core_ids=list(range(B)))
    global _last_exec_ns
    _last_exec_ns = res.exec_time_ns
    out = np.stack([r["out"] for r in res.results]).astype(np.float32)
    return out


_last_exec_ns = None
_last_nc = None


# revision 6
# speedup vs baseline: 1.7711x; 1.0002x over previous
# GGNN encoder kernel for Trainium2 (Bass/Tile), data-parallel over the
# batch dimension: 8 graphs -> 8 NeuronCores, one graph per core.
#
# Mixed-precision design ("R2"):
#  - r/z gates and gh_n run as fp8(e4m3) DoubleRow matmuls (two 128-deep
#    K-planes per instruction at 0.5 cycles/row).  Their quantization
#    noise is squashed by the sigmoid (and by r*gh_n being small).
#  - The sensitive n-gate input gi_n = A^T (h @ Wl @ Wih_n^T) is computed
#    exactly in f32r via a host-side weight fold (Wcn = Wl @ Wih_n^T),
#    which also deletes the separate per-layer m matmul from this path.
#  - h master stays fp32; embeddings/pooling/fusion run in bf16/f32r.
#  - All fp8 operands carry power-of-two scales, folded exactly into the
#    activation-function scale arguments.
#
# Per-core computation (one graph):
#   type_e  = type_table[node_types]                       # f32r one-hot matmul
#   tok_e   = word_emb[node_token_ids]                     # bf16 SWDGE dma_gather
#   text_e  = segment_mean(tok_e, token_seg_ids)           # bf16 pooling matmul
#   h       = concat(type_e, text_e) @ fusion_w + b        # bf16 matmul
#   4 x GGNN layer:
#     m8   = h8 @ Wl8          (fp8 DR)      -> agg8 = A8-contract (fp8 DR)
#     r,z  = sigmoid(Wih8 agg8 + Whh8 h8 + b)              (fp8 DR psum)
#     gin  = A-contract(h @ Wcn)                           (f32r, exact)
#     ghn  = Whh_n8 h8                                     (fp8 DR)
#     n    = tanh(gin + r*ghn + b);  h' = n + z*(h - n)    (fp32 pointwise)
#   out     = mask * h                                     # PE transpose epilogue

import functools

import numpy as np
import ml_dtypes

import concourse.bass as bass
import concourse.mybir as mybir
import concourse.tile as tile
from concourse import bacc, bass_utils
from concourse.masks import make_identity

# Problem shapes (hardcoded: kernel must be self-contained).
B, N, T, D, TD, L = 8, 512, 2048, 768, 128, 4
V, TYPES = 30522, 64
MAX_NODE_LEN = 512
F = TD + D            # 896 fused embedding dim
P = 128               # partitions
DCH = D // P          # 6 feature chunks
KK = DCH // 2         # 3 feature chunk pairs (DoubleRow planes)
NCH = N // P          # 4 node chunks
CC = NCH // 2         # 2 node chunk pairs
FCH = F // P          # 7 fused chunks
GCH = 3 * DCH         # 18 gate row chunks
TCH = T // P          # 16 token chunks
BLK = N // TCH        # 32 nodes per token chunk (block-pooling case)
NF = 512              # free-dim tile (nodes)
GS = 4                # token gather splits
GT = T // GS          # tokens per gather split (512)
GC = GT // P          # 128-chunks per gather split (4)

# power-of-two scale exponents for the fp8 operands
AH = 4                # h -> fp8
AM = 9                # m -> fp8
AG = 4                # agg -> fp8
BWL = 5               # ggnn_w
BIH = 5               # gru_w_ih (r,z rows)
BHH = 5               # gru_w_hh
SG = BIH + AG         # gate psum scale (== BHH + AH); Wcn also pre-scaled 2^SG
assert SG == BHH + AH

f32 = mybir.dt.float32
f32r = mybir.dt.float32r
bf16 = mybir.dt.bfloat16
fp8 = mybir.dt.float8e4
i16 = mybir.dt.int16
NPF8 = ml_dtypes.float8_e4m3
NPBF = ml_dtypes.bfloat16

Sigmoid = mybir.ActivationFunctionType.Sigmoid
Tanh = mybir.ActivationFunctionType.Tanh
Ident = mybir.ActivationFunctionType.Identity
DR = mybir.MatmulPerfMode.DoubleRow


def build_nc(pool_wide: bool, has_bhhn: bool) -> bass.Bass:
    nc = bacc.Bacc(num_swdge_queues=2, dynamic_dma_scratch_size=32768)

    tok_idx = nc.dram_tensor("tok_idx", [P, T // 16], i16, kind="ExternalInput")
    typ_oh = nc.dram_tensor("typ_oh", [TYPES, N], f32r, kind="ExternalInput")
    type_table = nc.dram_tensor("type_table", [TYPES, TD], f32r,
                                kind="ExternalInput")
    word_d = nc.dram_tensor("word_bf", [V, D], bf16, kind="ExternalInput")
    if pool_wide:
        poolm = nc.dram_tensor("poolw", [P, TCH, N], bf16, kind="ExternalInput")
    else:
        poolm = nc.dram_tensor("poolm", [P, TCH, BLK], bf16,
                               kind="ExternalInput")
    # fusion_w: [p, j, c, m] (bf16, one DMA)
    fusion_w = nc.dram_tensor("fusion_w", [P, DCH, FCH, P], bf16,
                              kind="ExternalInput")
    at_f = nc.dram_tensor("at_f", [P, NCH, N], bf16, kind="ExternalInput")
    at_8 = nc.dram_tensor("at_8", [P, CC, 2, N], fp8, kind="ExternalInput")
    wl8 = nc.dram_tensor("wl8", [L, P, KK, 2, D], fp8, kind="ExternalInput")
    wcn = nc.dram_tensor("wcn", [L, DCH, P, D], f32r, kind="ExternalInput")
    wih8 = nc.dram_tensor("wih8", [P, 2 * DCH, KK, 2, P], fp8,
                          kind="ExternalInput")
    whh8 = nc.dram_tensor("whh8", [P, GCH, KK, 2, P], fp8,
                          kind="ExternalInput")
    # smalls: [bsum(12) | bihn(6) | bhhn_s(6) | fusion_b(6) | mask(4)]
    smalls = nc.dram_tensor("smalls", [P, 34], f32, kind="ExternalInput")
    out = nc.dram_tensor("out", [N, D], f32, kind="ExternalOutput")

    wide = pool_wide
    with tile.TileContext(nc) as tc:
        with (
            tc.tile_pool(name="consts", bufs=1) as consts,
            tc.tile_pool(name="wts", bufs=1) as wts,
            tc.tile_pool(name="wlp", bufs=2) as wlp,
            tc.tile_pool(name="wcnp", bufs=6) as wcnp,
            tc.tile_pool(name="tokg", bufs=4) as tokg,
            tc.tile_pool(name="hp", bufs=12) as hp,
            tc.tile_pool(name="hf8", bufs=5 if wide else 6) as hf8,
            tc.tile_pool(name="mnp", bufs=4) as mnp,
            tc.tile_pool(name="mfp", bufs=5) as mfp,
            tc.tile_pool(name="aggp", bufs=4 if wide else 5) as aggp,
            tc.tile_pool(name="gp", bufs=11 if wide else 17) as gp,
            tc.tile_pool(name="obp", bufs=2) as obp,
            tc.tile_pool(name="ps5", bufs=6, space="PSUM") as ps5,
            tc.tile_pool(name="psN", bufs=2, space="PSUM") as psN,
        ):
            # ---- token gather first: it gates the whole front of the kernel
            tok_idx_sb = consts.tile([P, T // 16], i16)
            nc.sync.dma_start(out=tok_idx_sb[:], in_=tok_idx[:])
            if wide:
                pool_sb = consts.tile([P, TCH, N], bf16)
            else:
                pool_sb = consts.tile([P, TCH, BLK], bf16)
            nc.sync.dma_start(out=pool_sb[:], in_=poolm[:])

            tt_sb = consts.tile([TYPES, TD], f32r)
            nc.sync.dma_start(out=tt_sb[:], in_=type_table[:])
            oh_sb = consts.tile([TYPES, N], f32r)
            nc.sync.dma_start(out=oh_sb[:], in_=typ_oh[:])

            # fusion weights ahead of the token gathers on the DMA pipe:
            # fusion is the first consumer after pooling
            fw_sb = wts.tile([P, DCH, FCH, P], bf16)
            nc.scalar.dma_start(out=fw_sb[:], in_=fusion_w[:])
            wl_sb = wlp.tile([P, KK, 2, D], fp8, tag="wlp", name="wl0")
            nc.scalar.dma_start(out=wl_sb[:], in_=wl8[0])
            at8_sb = wts.tile([P, CC, 2, N], fp8)
            nc.scalar.dma_start(out=at8_sb[:], in_=at_8[:])

            gdt = bf16
            gath = []
            for s in range(GS):
                tg = tokg.tile([P, GC, D], gdt, tag="tokg", name=f"tokg{s}")
                nc.gpsimd.dma_gather(
                    tg[:], word_d[:],
                    tok_idx_sb[:, s * (GT // 16) : (s + 1) * (GT // 16)],
                    GT, GT, D, queue_num=s % 2,
                )
                gath.append(tg)

            # ---- remaining constants / small inputs ----
            identity = consts.tile([P, P], f32)
            make_identity(nc, identity[:])
            smalls_sb = consts.tile([P, 34], f32)
            nc.sync.dma_start(out=smalls_sb[:], in_=smalls[:])
            bsum_sb = smalls_sb[:, 0:12]
            bihn_sb = smalls_sb[:, 12:18]
            bhhn_sb = smalls_sb[:, 18:24]
            fb_sb = smalls_sb[:, 24:30]
            mask_sb = smalls_sb[:, 30:34]

            # remaining weights in the order the DMA pipe must serve them
            whh_sb = wts.tile([P, GCH, KK, 2, P], fp8)
            nc.sync.dma_start(out=whh_sb[:], in_=whh8[:])
            wih_sb = wts.tile([P, 2 * DCH, KK, 2, P], fp8)
            nc.scalar.dma_start(out=wih_sb[:], in_=wih8[:])
            wcn_sb = [
                wcnp.tile([P, D], f32r, tag="wcnp", name=f"wcn0_{k}")
                for k in range(DCH)
            ]
            for k in range(DCH):
                eng = nc.sync if k % 2 else nc.scalar
                eng.dma_start(out=wcn_sb[k][:], in_=wcn[0, k])
            atf_sb = wts.tile([P, NCH, N], bf16)
            nc.sync.dma_start(out=atf_sb[:], in_=at_f[:])

            # fused embedding, feature-major bf16: chunk 0 = type_e,
            # chunks 1..6 = text_e
            fusedT = [
                consts.tile([P, NF], bf16, name=f"fusedT{k}")
                for k in range(FCH)
            ]

            # type_eT = type_table.T @ onehot  (f32r, K=64)
            ptyp = ps5.tile([P, NF], f32, tag="ps5")
            nc.tensor.matmul(
                out=ptyp[:], lhsT=tt_sb[:], rhs=oh_sb[:], start=True, stop=True
            )
            nc.scalar.activation(out=fusedT[0][:], in_=ptyp[:], func=Ident,
                                 bias=0.0)

            # token pooling: 128 tokens -> 32 nodes (block) / 512 (wide)
            if wide:
                # c-major accumulation: each gather split is consumed once
                # and released (tokg ring holds only 2 splits in wide mode)
                pfs = [
                    ps5.tile([P, NF], f32, tag="ps5", name=f"pf{fch}")
                    for fch in range(DCH)
                ]
                for c in range(TCH):
                    s, c2 = divmod(c, GC)
                    for fch in range(DCH):
                        nc.tensor.matmul(
                            out=pfs[fch][:],
                            lhsT=gath[s][:, c2, fch * P : (fch + 1) * P],
                            rhs=pool_sb[:, c, :],
                            start=(c == 0), stop=(c == TCH - 1),
                        )
                for fch in range(DCH):
                    nc.scalar.activation(out=fusedT[1 + fch][:],
                                         in_=pfs[fch][:], func=Ident,
                                         bias=0.0)
            else:
                for fch in range(DCH):
                    pf = ps5.tile([P, NF], f32, tag="ps5", name=f"pf{fch}")
                    for c in range(TCH):
                        s, c2 = divmod(c, GC)
                        nc.tensor.matmul(
                            out=pf[:, c * BLK : (c + 1) * BLK],
                            lhsT=gath[s][:, c2, fch * P : (fch + 1) * P],
                            rhs=pool_sb[:, c, :],
                            start=True, stop=True,
                        )
                    nc.scalar.activation(out=fusedT[1 + fch][:], in_=pf[:],
                                         func=Ident, bias=0.0)

            # ---- fusion matmul: h0 feature-major f32 + fp8 shadow ----
            hT = []
            hF = [
                hf8.tile([P, 2, NF], fp8, tag="hf8", name=f"h0q{kk}")
                for kk in range(KK)
            ]
            for j in range(DCH):
                pfj = ps5.tile([P, NF], f32, tag="ps5")
                for c in range(FCH):
                    nc.tensor.matmul(
                        out=pfj[:],
                        lhsT=fw_sb[:, j, c, :],
                        rhs=fusedT[c][:],
                        start=(c == 0), stop=(c == FCH - 1),
                    )
                hj = hp.tile([P, NF], f32r, tag="hp", name=f"h0_{j}")
                nc.scalar.activation(
                    out=hj[:], in_=pfj[:], func=Ident,
                    bias=fb_sb[:, j : j + 1],
                )
                hT.append(hj)
                nc.vector.tensor_scalar_mul(
                    out=hF[j // 2][:, j % 2, :], in0=hj[:],
                    scalar1=float(2 ** AH),
                )

            # ---- GGNN layers ----
            for l in range(L):
                # m8 = h8 @ Wl8 (fp8 DR) and m' = h @ Wcn (f32r, exact),
                # both node-major, chunk-interleaved so the f32r m' work
                # covers the psum->sbuf copy latency of m8
                mN = [
                    mnp.tile([P, 2, D], fp8, tag="mnp", name=f"mN{l}_{cc}")
                    for cc in range(CC)
                ]
                mpN = [
                    mfp.tile([P, D], bf16, tag="mfp", name=f"mp{l}_{i}")
                    for i in range(NCH)
                ]

                def emit_m8(i):
                    pma = ps5.tile([P, NF], f32, tag="ps5", name=f"pma{l}_{i}")
                    pmb = ps5.tile([P, D - NF], f32, tag="ps5",
                                   name=f"pmb{l}_{i}")
                    for kk in range(KK):
                        lh = hF[kk][:, :, i * P : (i + 1) * P]
                        nc.tensor.matmul(
                            out=pma[:], lhsT=lh, rhs=wl_sb[:, kk, :, :NF],
                            start=(kk == 0), stop=(kk == KK - 1), perf_mode=DR,
                        )
                        nc.tensor.matmul(
                            out=pmb[:], lhsT=lh, rhs=wl_sb[:, kk, :, NF:D],
                            start=(kk == 0), stop=(kk == KK - 1), perf_mode=DR,
                        )
                    nc.scalar.activation(
                        out=mN[i // 2][:, i % 2, :NF], in_=pma[:],
                        func=Ident, bias=0.0,
                    )
                    nc.vector.tensor_copy(
                        out=mN[i // 2][:, i % 2, NF:D], in_=pmb[:]
                    )

                def emit_mp(i):
                    pca = ps5.tile([P, NF], f32, tag="ps5", name=f"pca{l}_{i}")
                    pcb = ps5.tile([P, D - NF], f32, tag="ps5",
                                   name=f"pcb{l}_{i}")
                    for k in range(DCH):
                        lh = hT[k][:, i * P : (i + 1) * P]
                        nc.tensor.matmul(
                            out=pca[:], lhsT=lh, rhs=wcn_sb[k][:, :NF],
                            start=(k == 0), stop=(k == DCH - 1),
                        )
                        nc.tensor.matmul(
                            out=pcb[:], lhsT=lh, rhs=wcn_sb[k][:, NF:D],
                            start=(k == 0), stop=(k == DCH - 1),
                        )
                    nc.scalar.activation(out=mpN[i][:, :NF], in_=pca[:],
                                         func=Ident, bias=0.0)
                    nc.vector.tensor_copy(out=mpN[i][:, NF:D], in_=pcb[:])

                # m' first: it reads the f32 master h directly (no
                # dependence on the fp8 quantize tail of the previous layer);
                # contract k=0..3 first so the PE has work before the last
                # h chunks of the previous layer finish
                mp_ps = {}
                for i in range(NCH):
                    pca = ps5.tile([P, NF], f32, tag="ps5",
                                   name=f"pca{l}_{i}")
                    pcb = ps5.tile([P, D - NF], f32, tag="ps5",
                                   name=f"pcb{l}_{i}")
                    mp_ps[i] = (pca, pcb)
                    for k in range(4):
                        lh = hT[k][:, i * P : (i + 1) * P]
                        nc.tensor.matmul(
                            out=pca[:], lhsT=lh, rhs=wcn_sb[k][:, :NF],
                            start=(k == 0), stop=False,
                        )
                        nc.tensor.matmul(
                            out=pcb[:], lhsT=lh, rhs=wcn_sb[k][:, NF:D],
                            start=(k == 0), stop=False,
                        )
                    if i == 2 or i == 3:
                        for j in ((0, 1, 2) if i == 2 else (3,)):
                            pca_, pcb_ = mp_ps[j]
                            for k in range(4, DCH):
                                lh = hT[k][:, j * P : (j + 1) * P]
                                nc.tensor.matmul(
                                    out=pca_[:], lhsT=lh,
                                    rhs=wcn_sb[k][:, :NF],
                                    start=False, stop=(k == DCH - 1),
                                )
                                nc.tensor.matmul(
                                    out=pcb_[:], lhsT=lh,
                                    rhs=wcn_sb[k][:, NF:D],
                                    start=False, stop=(k == DCH - 1),
                                )
                            nc.scalar.activation(out=mpN[j][:, :NF],
                                                 in_=pca_[:], func=Ident,
                                                 bias=0.0)
                            nc.vector.tensor_copy(out=mpN[j][:, NF:D],
                                                  in_=pcb_[:])
                for i in range(NCH):
                    emit_m8(i)

                # pre-open r/z psum groups with their gh contributions so
                # the PE has work while agg8 quantizes drain
                rz_ps = {}

                def emit_gh_rz(i):
                    prs = []
                    for g in (i, DCH + i):
                        pg = ps5.tile([P, NF], f32, tag="ps5",
                                      name=f"prz{l}_{g}")
                        for kk in range(KK):
                            nc.tensor.matmul(
                                out=pg[:], lhsT=whh_sb[:, g, kk, :, :],
                                rhs=hF[kk][:],
                                start=(kk == 0), stop=False, perf_mode=DR,
                            )
                        prs.append(pg)
                    rz_ps[i] = prs

                emit_gh_rz(0)

                # agg8 = A8-contraction of m8 (fp8 DR), feature-major
                aggF = [
                    aggp.tile([P, 2, NF], fp8, tag="aggp", name=f"agg{l}_{kk}")
                    for kk in range(KK)
                ]
                for fch in range(DCH):
                    pag = ps5.tile([P, NF], f32, tag="ps5")
                    for cc in range(CC):
                        nc.tensor.matmul(
                            out=pag[:],
                            lhsT=mN[cc][:, :, fch * P : (fch + 1) * P],
                            rhs=at8_sb[:, cc, :, :],
                            start=(cc == 0), stop=(cc == CC - 1), perf_mode=DR,
                        )
                    if fch % 2 == 0:
                        nc.scalar.activation(
                            out=aggF[fch // 2][:, fch % 2, :], in_=pag[:],
                            func=Ident, bias=0.0,
                            scale=float(2.0 ** (AG - AM)),
                        )
                    else:
                        nc.vector.tensor_scalar_mul(
                            out=aggF[fch // 2][:, fch % 2, :], in0=pag[:],
                            scalar1=float(2.0 ** (AG - AM)),
                        )

                # prefetch next layer's weights while gates run
                if l + 1 < L:
                    wl_next = wlp.tile([P, KK, 2, D], fp8, tag="wlp",
                                       name=f"wl{l + 1}")
                    nc.scalar.dma_start(out=wl_next[:], in_=wl8[l + 1])
                    wcn_next = [
                        wcnp.tile([P, D], f32r, tag="wcnp",
                                  name=f"wcn{l + 1}_{k}")
                        for k in range(DCH)
                    ]
                    for k in range(DCH):
                        eng = nc.sync if k % 2 else nc.scalar
                        eng.dma_start(out=wcn_next[k][:], in_=wcn[l + 1, k])

                # GRU gates, 128 gate rows at a time (feature chunk i)
                hTn = []
                hFn = [
                    hf8.tile([P, 2, NF], fp8, tag="hf8",
                             name=f"h{l + 1}q{kk}")
                    for kk in range(KK)
                ] if l + 1 < L else []
                # stage-skewed gate pipeline: A(i) = matmuls + sigmoids,
                # B(i) = rn/tn/tanh, C(i) = h' update + fp8 shadow.  Skewing
                # keeps every engine's in-order queue dependency-ready.
                st = [dict() for _ in range(DCH)]

                def stageA(i):
                    rz = []
                    for gi_, (g, pg) in enumerate(
                        ((i, rz_ps[i][0]), (DCH + i, rz_ps[i][1]))
                    ):
                        for kk in range(KK):
                            nc.tensor.matmul(
                                out=pg[:], lhsT=wih_sb[:, g, kk, :, :],
                                rhs=aggF[kk][:],
                                start=False, stop=(kk == KK - 1), perf_mode=DR,
                            )
                        gs = gp.tile([P, NF], f32, tag="gp",
                                     name=f"g{l}_{i}_{gi_}")
                        nc.scalar.activation(
                            out=gs[:], in_=pg[:], func=Sigmoid,
                            bias=bsum_sb[:, g : g + 1],
                            scale=float(2.0 ** -SG),
                        )
                        rz.append(gs)
                    del rz_ps[i]
                    if i + 1 < DCH and i + 1 not in rz_ps:
                        emit_gh_rz(i + 1)
                    st[i]["r"], st[i]["z"] = rz

                def stageB(i):
                    s_i = st[i]
                    g = 2 * DCH + i
                    pghn = psN.tile([P, NF], f32, tag="psN",
                                    name=f"pghn{l}_{i}")
                    for kk in range(KK):
                        nc.tensor.matmul(
                            out=pghn[:], lhsT=whh_sb[:, g, kk, :, :],
                            rhs=hF[kk][:],
                            start=(kk == 0), stop=(kk == KK - 1), perf_mode=DR,
                        )
                    pgin = psN.tile([P, NF], f32, tag="psN",
                                    name=f"pgin{l}_{i}")
                    for c in range(NCH):
                        nc.tensor.matmul(
                            out=pgin[:],
                            lhsT=mpN[c][:, i * P : (i + 1) * P],
                            rhs=atf_sb[:, c, :],
                            start=(c == 0), stop=(c == NCH - 1),
                        )
                    s_i["pghn"], s_i["pgin"] = pghn, pgin
                    rn = gp.tile([P, NF], f32, tag="gp", name=f"rn{l}_{i}")
                    if has_bhhn:
                        hb = gp.tile([P, NF], f32, tag="gp",
                                     name=f"hb{l}_{i}")
                        nc.scalar.activation(
                            out=hb[:], in_=s_i["pghn"][:], func=Ident,
                            bias=bhhn_sb[:, i : i + 1], scale=1.0,
                        )
                        nc.vector.tensor_mul(out=rn[:], in0=s_i["r"][:],
                                             in1=hb[:])
                    else:
                        nc.vector.tensor_mul(out=rn[:], in0=s_i["r"][:],
                                             in1=s_i["pghn"][:])
                    tn = gp.tile([P, NF], f32, tag="gp", name=f"tn{l}_{i}")
                    nc.vector.tensor_add(out=tn[:], in0=s_i["pgin"][:],
                                         in1=rn[:])
                    nn_ = gp.tile([P, NF], f32, tag="gp", name=f"nn{l}_{i}")
                    nc.scalar.activation(
                        out=nn_[:], in_=tn[:], func=Tanh,
                        bias=bihn_sb[:, i : i + 1], scale=float(2.0 ** -SG),
                    )
                    s_i["n"] = nn_

                def stageC(i):
                    s_i = st[i]
                    nn_ = s_i["n"]
                    s_ = gp.tile([P, NF], f32, tag="gp", name=f"s{l}_{i}")
                    nc.gpsimd.tensor_sub(out=s_[:], in0=hT[i][:], in1=nn_[:])
                    sz = gp.tile([P, NF], f32, tag="gp", name=f"sz{l}_{i}")
                    nc.gpsimd.tensor_mul(out=sz[:], in0=s_i["z"][:], in1=s_[:])
                    hj = hp.tile([P, NF], f32r, tag="hp",
                                 name=f"h{l + 1}_{i}")
                    nc.vector.tensor_add(out=hj[:], in0=nn_[:], in1=sz[:])
                    hTn.append(hj)
                    if l + 1 < L:
                        if i % 2 == 0:
                            nc.scalar.activation(
                                out=hFn[i // 2][:, i % 2, :], in_=hj[:],
                                func=Ident, bias=0.0, scale=float(2 ** AH),
                            )
                        else:
                            nc.vector.tensor_scalar_mul(
                                out=hFn[i // 2][:, i % 2, :], in0=hj[:],
                                scalar1=float(2 ** AH),
                            )

                for t in range(DCH + 2):
                    if t < DCH:
                        stageA(t)
                    if 1 <= t + 0 and 0 <= t - 1 < DCH:
                        stageB(t - 1)
                    if 0 <= t - 2 < DCH:
                        stageC(t - 2)
                hT, hF = hTn, hFn
                if l + 1 < L:
                    wl_sb = wl_next
                    wcn_sb = wcn_next

            # ---- transpose back to node-major, mask, write out ----
            # j-major: transposes for feature chunk j start as soon as the
            # last layer's h'(j) lands, overlapping the pointwise drain
            poas = [ps5.tile([P, NF], f32, tag="ps5", name=f"poa{i}")
                    for i in range(NCH)]
            pobs = [psN.tile([P, D - NF], f32, tag="psN", name=f"pob{i}")
                    for i in range(NCH)]
            for j in range(DCH):
                for i in range(NCH):
                    dst = poas[i][:, j * P : (j + 1) * P] if j < 4 else \
                        pobs[i][:, (j - 4) * P : (j - 3) * P]
                    nc.tensor.transpose(
                        out=dst,
                        in_=hT[j][:, i * P : (i + 1) * P].bitcast(f32),
                        identity=identity[:],
                    )
            for i in range(NCH):
                ob = obp.tile([P, D], f32, tag="obp")
                nc.vector.tensor_scalar_mul(
                    out=ob[:, :NF], in0=poas[i][:],
                    scalar1=mask_sb[:, i : i + 1],
                )
                nc.scalar.activation(
                    out=ob[:, NF:D], in_=pobs[i][:], func=Ident, bias=0.0,
                    scale=mask_sb[:, i : i + 1],
                )
                eng = nc.sync if i % 2 else nc.scalar
                eng.dma_start(out=out[i * P : (i + 1) * P, :NF],
                              in_=ob[:, :NF])
                eng2 = nc.scalar if i % 2 else nc.sync
                eng2.dma_start(out=out[i * P : (i + 1) * P, NF:D],
                               in_=ob[:, NF:D])

    nc.compile()
    return nc


@functools.lru_cache(maxsize=4)
def _get_nc(pool_wide: bool, has_bhhn: bool) -> bass.Bass:
    return build_nc(pool_wide, has_bhhn)


def _prep_shared(inputs, pool_wide):
    """Weight tensors identical across graphs, pre-quantized / pre-laid-out
    partition-major so every DMA is contiguous per partition."""
    word = np.asarray(inputs["word_emb"], np.float32)
    tt = np.ascontiguousarray(np.asarray(inputs["type_table"], np.float32))
    fw = np.asarray(inputs["fusion_w"], np.float32)          # [F, D]
    # [p, j, c, m] = fw[c*128+p, j*128+m]
    fusion_w = np.ascontiguousarray(
        fw.reshape(FCH, P, DCH, P).transpose(1, 2, 0, 3)
    ).astype(NPBF)
    wl_w = np.asarray(inputs["ggnn_w"], np.float32)          # [L, D, D]
    wl8 = np.ascontiguousarray(
        (wl_w * 2.0 ** BWL).reshape(L, KK, 2, P, D).transpose(0, 3, 1, 2, 4)
    ).astype(NPF8)
    wih_w = np.asarray(inputs["gru_w_ih"], np.float32)       # [3D, D]
    whh_w = np.asarray(inputs["gru_w_hh"], np.float32)
    # [p, g, kk, i, m] = W.T[(2kk+i)*128+p, g*128+m] * scale
    wih8 = np.ascontiguousarray(
        (wih_w[: 2 * D].T * 2.0 ** BIH)
        .reshape(KK, 2, P, 2 * DCH, P).transpose(2, 3, 0, 1, 4)
    ).astype(NPF8)
    whh8 = np.ascontiguousarray(
        (whh_w.T * 2.0 ** BHH)
        .reshape(KK, 2, P, GCH, P).transpose(2, 3, 0, 1, 4)
    ).astype(NPF8)
    # Wcn_l = Wl @ Wih_n^T, pre-scaled by 2^SG; [l, k, p, j]
    wih_n = wih_w[2 * D :]
    wcn = np.stack([
        (wl_w[l].astype(np.float64) @ wih_n.T.astype(np.float64))
        for l in range(L)
    ]).astype(np.float32) * 2.0 ** SG
    wcn = np.ascontiguousarray(wcn.reshape(L, DCH, P, D))
    bih = np.asarray(inputs["gru_b_ih"], np.float32)
    bhh = np.asarray(inputs["gru_b_hh"], np.float32)
    smalls = np.zeros((P, 34), np.float32)
    smalls[:, 0:12] = (bih[: 2 * D] + bhh[: 2 * D]).reshape(2 * DCH, P).T
    smalls[:, 12:18] = bih[2 * D :].reshape(DCH, P).T
    smalls[:, 18:24] = (bhh[2 * D :] * 2.0 ** SG).reshape(DCH, P).T
    smalls[:, 24:30] = (
        np.asarray(inputs["fusion_b"], np.float32).reshape(DCH, P).T
    )
    shared = dict(
        type_table=tt, fusion_w=fusion_w, wl8=wl8, wcn=wcn, wih8=wih8,
        whh8=whh8, _smalls_base=smalls,
    )
    shared["word_bf"] = np.ascontiguousarray(word.astype(NPBF))
    return shared


def _graph_blockable(inputs, b):
    seg = np.asarray(inputs["token_seg_ids"][b], np.int64)
    tcol = np.arange(T) // P
    return bool(np.all((seg >= tcol * BLK) & (seg < (tcol + 1) * BLK)))


def _prep_graph(inputs, b, pool_wide):
    tok = np.asarray(inputs["node_token_ids"][b], np.int64)
    typ = np.asarray(inputs["node_types"][b], np.int32)
    seg = np.asarray(inputs["token_seg_ids"][b], np.int64)
    lens = np.asarray(inputs["node_token_lens"][b], np.float64)
    glen = int(np.asarray(inputs["graph_node_lens"][b]))
    esrc = np.asarray(inputs["edge_src"][b], np.int64)
    edst = np.asarray(inputs["edge_dst"][b], np.int64)
    ew = np.asarray(inputs["edge_weight"][b], np.float32)

    # token idxs for dma_gather: GS splits of GT idxs, each wrapped into
    # 16 partitions and replicated to 128 partitions
    tok16 = tok.astype(np.int16)
    cols = []
    for s in range(GS):
        w16 = tok16[s * GT : (s + 1) * GT].reshape(GT // 16, 16).T
        cols.append(np.tile(w16, (8, 1)))
    tok_idx = np.ascontiguousarray(np.concatenate(cols, axis=1))

    typ_oh = np.zeros((TYPES, N), np.float32)
    typ_oh[typ, np.arange(N)] = 1.0

    # dense adjacency A[src, dst]: f32r copy + DoubleRow-paired fp8 copy
    at = np.zeros((N, N), np.float32)
    np.add.at(at, (esrc, edst), ew)
    at_f = np.ascontiguousarray(
        at.reshape(NCH, P, N).transpose(1, 0, 2)
    ).astype(NPBF)
    at_8 = np.ascontiguousarray(
        at.reshape(CC, 2, P, N).transpose(2, 0, 1, 3)
    ).astype(NPF8)

    winv = np.zeros(N, np.float64)
    nzmask = lens != 0
    winv[nzmask] = 1.0 / lens[nzmask]
    t_ = np.arange(T)
    c_ = t_ // P
    if pool_wide:
        pm = np.zeros((TCH, P, N), np.float32)
        pm[c_, t_ % P, seg] = winv[seg]
        pool_arr = np.ascontiguousarray(pm.transpose(1, 0, 2)).astype(NPBF)
        pool_key = "poolw"
    else:
        pm = np.zeros((TCH, P, BLK), np.float32)
        pm[c_, t_ % P, seg - c_ * BLK] = winv[seg]
        pool_arr = np.ascontiguousarray(pm.transpose(1, 0, 2)).astype(NPBF)
        pool_key = "poolm"

    keep = min(glen, MAX_NODE_LEN)
    mask = np.ascontiguousarray(
        (np.arange(N) < keep).astype(np.float32).reshape(NCH, P).T
    )
    return {
        "tok_idx": tok_idx, "typ_oh": typ_oh, "at_f": at_f, "at_8": at_8,
        pool_key: pool_arr, "_mask": mask,
    }


def kernel(**inputs) -> np.ndarray:
    pool_wide = not all(_graph_blockable(inputs, b) for b in range(B))
    has_bhhn = bool(
        np.any(np.asarray(inputs["gru_b_hh"], np.float32)[2 * D :] != 0.0)
    )
    shared = _prep_shared(inputs, pool_wide)
    smalls_base = shared.pop("_smalls_base")
    per_graph = [_prep_graph(inputs, b, pool_wide) for b in range(B)]
    in_maps = []
    for b in range(B):
        g = dict(per_graph[b])
        sm = smalls_base.copy()
        sm[:, 30:34] = g.pop("_mask")
        g["smalls"] = sm
        in_maps.append({**shared, **g})
    nc = _get_nc(pool_wide, has_bhhn)
    res = bass_utils.run_bass_kernel_spmd(nc, in_maps, core_ids=list(range(B)))
    global _last_exec_ns
    _last_exec_ns = res.exec_time_ns
    out = np.stack([r["out"] for r in res.results]).astype(np.float32)
    return out


_last_exec_ns = None
_last_nc = None


# revision 7
# speedup vs baseline: 1.7794x; 1.0047x over previous
# GGNN encoder kernel for Trainium2 (Bass/Tile), data-parallel over the
# batch dimension: 8 graphs -> 8 NeuronCores, one graph per core.
#
# Mixed-precision design ("R2"):
#  - r/z gates and gh_n run as fp8(e4m3) DoubleRow matmuls (two 128-deep
#    K-planes per instruction at 0.5 cycles/row).  Their quantization
#    noise is squashed by the sigmoid (and by r*gh_n being small).
#  - The sensitive n-gate input gi_n = A^T (h @ Wl @ Wih_n^T) is computed
#    exactly in f32r via a host-side weight fold (Wcn = Wl @ Wih_n^T),
#    which also deletes the separate per-layer m matmul from this path.
#  - h master stays fp32; embeddings/pooling/fusion run in bf16/f32r.
#  - All fp8 operands carry power-of-two scales, folded exactly into the
#    activation-function scale arguments.
#
# Per-core computation (one graph):
#   type_e  = type_table[node_types]                       # f32r one-hot matmul
#   tok_e   = word_emb[node_token_ids]                     # bf16 SWDGE dma_gather
#   text_e  = segment_mean(tok_e, token_seg_ids)           # bf16 pooling matmul
#   h       = concat(type_e, text_e) @ fusion_w + b        # bf16 matmul
#   4 x GGNN layer:
#     m8   = h8 @ Wl8          (fp8 DR)      -> agg8 = A8-contract (fp8 DR)
#     r,z  = sigmoid(Wih8 agg8 + Whh8 h8 + b)              (fp8 DR psum)
#     gin  = A-contract(h @ Wcn)                           (f32r, exact)
#     ghn  = Whh_n8 h8                                     (fp8 DR)
#     n    = tanh(gin + r*ghn + b);  h' = n + z*(h - n)    (fp32 pointwise)
#   out     = mask * h                                     # PE transpose epilogue

import functools

import numpy as np
import ml_dtypes

import concourse.bass as bass
import concourse.mybir as mybir
import concourse.tile as tile
from concourse import bacc, bass_utils
from concourse.masks import make_identity

# Problem shapes (hardcoded: kernel must be self-contained).
B, N, T, D, TD, L = 8, 512, 2048, 768, 128, 4
V, TYPES = 30522, 64
MAX_NODE_LEN = 512
F = TD + D            # 896 fused embedding dim
P = 128               # partitions
DCH = D // P          # 6 feature chunks
KK = DCH // 2         # 3 feature chunk pairs (DoubleRow planes)
NCH = N // P          # 4 node chunks
CC = NCH // 2         # 2 node chunk pairs
FCH = F // P          # 7 fused chunks
GCH = 3 * DCH         # 18 gate row chunks
TCH = T // P          # 16 token chunks
BLK = N // TCH        # 32 nodes per token chunk (block-pooling case)
NF = 512              # free-dim tile (nodes)
GS = 2                # token gather splits
GT = T // GS          # tokens per gather split (512)
GC = GT // P          # 128-chunks per gather split (4)

# power-of-two scale exponents for the fp8 operands
AH = 4                # h -> fp8
AM = 9                # m -> fp8
AG = 4                # agg -> fp8
BWL = 5               # ggnn_w
BIH = 5               # gru_w_ih (r,z rows)
BHH = 5               # gru_w_hh
SG = BIH + AG         # gate psum scale (== BHH + AH); Wcn also pre-scaled 2^SG
assert SG == BHH + AH

f32 = mybir.dt.float32
f32r = mybir.dt.float32r
bf16 = mybir.dt.bfloat16
fp8 = mybir.dt.float8e4
i16 = mybir.dt.int16
NPF8 = ml_dtypes.float8_e4m3
NPBF = ml_dtypes.bfloat16

Sigmoid = mybir.ActivationFunctionType.Sigmoid
Tanh = mybir.ActivationFunctionType.Tanh
Ident = mybir.ActivationFunctionType.Identity
DR = mybir.MatmulPerfMode.DoubleRow


def build_nc(pool_wide: bool, has_bhhn: bool) -> bass.Bass:
    nc = bacc.Bacc(num_swdge_queues=2, dynamic_dma_scratch_size=32768)

    tok_idx = nc.dram_tensor("tok_idx", [P, T // 16], i16, kind="ExternalInput")
    typ_oh = nc.dram_tensor("typ_oh", [TYPES, N], f32r, kind="ExternalInput")
    type_table = nc.dram_tensor("type_table", [TYPES, TD], f32r,
                                kind="ExternalInput")
    word_d = nc.dram_tensor("word_bf", [V, D], bf16, kind="ExternalInput")
    if pool_wide:
        poolm = nc.dram_tensor("poolw", [P, TCH, N], bf16, kind="ExternalInput")
    else:
        poolm = nc.dram_tensor("poolm", [P, TCH, BLK], bf16,
                               kind="ExternalInput")
    # fusion_w: [p, j, c, m] (bf16, one DMA)
    fusion_w = nc.dram_tensor("fusion_w", [P, DCH, FCH, P], bf16,
                              kind="ExternalInput")
    at_f = nc.dram_tensor("at_f", [P, NCH, N], bf16, kind="ExternalInput")
    at_8 = nc.dram_tensor("at_8", [P, CC, 2, N], fp8, kind="ExternalInput")
    wl8 = nc.dram_tensor("wl8", [L, P, KK, 2, D], fp8, kind="ExternalInput")
    wcn = nc.dram_tensor("wcn", [L, DCH, P, D], f32r, kind="ExternalInput")
    wih8 = nc.dram_tensor("wih8", [P, 2 * DCH, KK, 2, P], fp8,
                          kind="ExternalInput")
    whh8 = nc.dram_tensor("whh8", [P, GCH, KK, 2, P], fp8,
                          kind="ExternalInput")
    # smalls: [bsum(12) | bihn(6) | bhhn_s(6) | fusion_b(6) | mask(4)]
    smalls = nc.dram_tensor("smalls", [P, 34], f32, kind="ExternalInput")
    out = nc.dram_tensor("out", [N, D], f32, kind="ExternalOutput")

    wide = pool_wide
    with tile.TileContext(nc) as tc:
        with (
            tc.tile_pool(name="consts", bufs=1) as consts,
            tc.tile_pool(name="wts", bufs=1) as wts,
            tc.tile_pool(name="wlp", bufs=2) as wlp,
            tc.tile_pool(name="wcnp", bufs=6) as wcnp,
            tc.tile_pool(name="tokg", bufs=2) as tokg,
            tc.tile_pool(name="hp", bufs=12) as hp,
            tc.tile_pool(name="hf8", bufs=5 if wide else 6) as hf8,
            tc.tile_pool(name="mnp", bufs=4) as mnp,
            tc.tile_pool(name="mfp", bufs=5) as mfp,
            tc.tile_pool(name="aggp", bufs=4 if wide else 5) as aggp,
            tc.tile_pool(name="gp", bufs=11 if wide else 17) as gp,
            tc.tile_pool(name="obp", bufs=2) as obp,
            tc.tile_pool(name="ps5", bufs=6, space="PSUM") as ps5,
            tc.tile_pool(name="psN", bufs=2, space="PSUM") as psN,
        ):
            # ---- token gather first: it gates the whole front of the kernel
            tok_idx_sb = consts.tile([P, T // 16], i16)
            nc.sync.dma_start(out=tok_idx_sb[:], in_=tok_idx[:])
            if wide:
                pool_sb = consts.tile([P, TCH, N], bf16)
            else:
                pool_sb = consts.tile([P, TCH, BLK], bf16)
            nc.sync.dma_start(out=pool_sb[:], in_=poolm[:])

            tt_sb = consts.tile([TYPES, TD], f32r)
            nc.sync.dma_start(out=tt_sb[:], in_=type_table[:])
            oh_sb = consts.tile([TYPES, N], f32r)
            nc.sync.dma_start(out=oh_sb[:], in_=typ_oh[:])

            # fusion weights ahead of the token gathers on the DMA pipe:
            # fusion is the first consumer after pooling
            fw_sb = wts.tile([P, DCH, FCH, P], bf16)
            nc.scalar.dma_start(out=fw_sb[:], in_=fusion_w[:])
            wl_sb = wlp.tile([P, KK, 2, D], fp8, tag="wlp", name="wl0")
            nc.scalar.dma_start(out=wl_sb[:], in_=wl8[0])
            at8_sb = wts.tile([P, CC, 2, N], fp8)
            nc.scalar.dma_start(out=at8_sb[:], in_=at_8[:])

            gdt = bf16
            gath = []
            for s in range(GS):
                tg = tokg.tile([P, GC, D], gdt, tag="tokg", name=f"tokg{s}")
                nc.gpsimd.dma_gather(
                    tg[:], word_d[:],
                    tok_idx_sb[:, s * (GT // 16) : (s + 1) * (GT // 16)],
                    GT, GT, D, queue_num=s % 2,
                )
                gath.append(tg)

            # ---- remaining constants / small inputs ----
            identity = consts.tile([P, P], f32)
            make_identity(nc, identity[:])
            smalls_sb = consts.tile([P, 34], f32)
            nc.sync.dma_start(out=smalls_sb[:], in_=smalls[:])
            bsum_sb = smalls_sb[:, 0:12]
            bihn_sb = smalls_sb[:, 12:18]
            bhhn_sb = smalls_sb[:, 18:24]
            fb_sb = smalls_sb[:, 24:30]
            mask_sb = smalls_sb[:, 30:34]

            # remaining weights in the order the DMA pipe must serve them
            whh_sb = wts.tile([P, GCH, KK, 2, P], fp8)
            nc.sync.dma_start(out=whh_sb[:], in_=whh8[:])
            wih_sb = wts.tile([P, 2 * DCH, KK, 2, P], fp8)
            nc.scalar.dma_start(out=wih_sb[:], in_=wih8[:])
            wcn_sb = [
                wcnp.tile([P, D], f32r, tag="wcnp", name=f"wcn0_{k}")
                for k in range(DCH)
            ]
            for k in range(DCH):
                eng = nc.sync if k % 2 else nc.scalar
                eng.dma_start(out=wcn_sb[k][:], in_=wcn[0, k])
            atf_sb = wts.tile([P, NCH, N], bf16)
            nc.sync.dma_start(out=atf_sb[:], in_=at_f[:])

            # fused embedding, feature-major bf16: chunk 0 = type_e,
            # chunks 1..6 = text_e
            fusedT = [
                consts.tile([P, NF], bf16, name=f"fusedT{k}")
                for k in range(FCH)
            ]

            # type_eT = type_table.T @ onehot  (f32r, K=64)
            ptyp = ps5.tile([P, NF], f32, tag="ps5")
            nc.tensor.matmul(
                out=ptyp[:], lhsT=tt_sb[:], rhs=oh_sb[:], start=True, stop=True
            )
            nc.scalar.activation(out=fusedT[0][:], in_=ptyp[:], func=Ident,
                                 bias=0.0)

            # token pooling: 128 tokens -> 32 nodes (block) / 512 (wide)
            if wide:
                # c-major accumulation: each gather split is consumed once
                # and released (tokg ring holds only 2 splits in wide mode)
                pfs = [
                    ps5.tile([P, NF], f32, tag="ps5", name=f"pf{fch}")
                    for fch in range(DCH)
                ]
                for c in range(TCH):
                    s, c2 = divmod(c, GC)
                    for fch in range(DCH):
                        nc.tensor.matmul(
                            out=pfs[fch][:],
                            lhsT=gath[s][:, c2, fch * P : (fch + 1) * P],
                            rhs=pool_sb[:, c, :],
                            start=(c == 0), stop=(c == TCH - 1),
                        )
                for fch in range(DCH):
                    nc.scalar.activation(out=fusedT[1 + fch][:],
                                         in_=pfs[fch][:], func=Ident,
                                         bias=0.0)
            else:
                for fch in range(DCH):
                    pf = ps5.tile([P, NF], f32, tag="ps5", name=f"pf{fch}")
                    for c in range(TCH):
                        s, c2 = divmod(c, GC)
                        nc.tensor.matmul(
                            out=pf[:, c * BLK : (c + 1) * BLK],
                            lhsT=gath[s][:, c2, fch * P : (fch + 1) * P],
                            rhs=pool_sb[:, c, :],
                            start=True, stop=True,
                        )
                    nc.scalar.activation(out=fusedT[1 + fch][:], in_=pf[:],
                                         func=Ident, bias=0.0)

            # ---- fusion matmul: h0 feature-major f32 + fp8 shadow ----
            hT = []
            hF = [
                hf8.tile([P, 2, NF], fp8, tag="hf8", name=f"h0q{kk}")
                for kk in range(KK)
            ]
            for j in range(DCH):
                pfj = ps5.tile([P, NF], f32, tag="ps5")
                for c in range(FCH):
                    nc.tensor.matmul(
                        out=pfj[:],
                        lhsT=fw_sb[:, j, c, :],
                        rhs=fusedT[c][:],
                        start=(c == 0), stop=(c == FCH - 1),
                    )
                hj = hp.tile([P, NF], f32r, tag="hp", name=f"h0_{j}")
                nc.scalar.activation(
                    out=hj[:], in_=pfj[:], func=Ident,
                    bias=fb_sb[:, j : j + 1],
                )
                hT.append(hj)
                nc.vector.tensor_scalar_mul(
                    out=hF[j // 2][:, j % 2, :], in0=hj[:],
                    scalar1=float(2 ** AH),
                )

            # ---- GGNN layers ----
            for l in range(L):
                # m8 = h8 @ Wl8 (fp8 DR) and m' = h @ Wcn (f32r, exact),
                # both node-major, chunk-interleaved so the f32r m' work
                # covers the psum->sbuf copy latency of m8
                mN = [
                    mnp.tile([P, 2, D], fp8, tag="mnp", name=f"mN{l}_{cc}")
                    for cc in range(CC)
                ]
                mpN = [
                    mfp.tile([P, D], bf16, tag="mfp", name=f"mp{l}_{i}")
                    for i in range(NCH)
                ]

                def emit_m8(i):
                    pma = ps5.tile([P, NF], f32, tag="ps5", name=f"pma{l}_{i}")
                    pmb = ps5.tile([P, D - NF], f32, tag="ps5",
                                   name=f"pmb{l}_{i}")
                    for kk in range(KK):
                        lh = hF[kk][:, :, i * P : (i + 1) * P]
                        nc.tensor.matmul(
                            out=pma[:], lhsT=lh, rhs=wl_sb[:, kk, :, :NF],
                            start=(kk == 0), stop=(kk == KK - 1), perf_mode=DR,
                        )
                        nc.tensor.matmul(
                            out=pmb[:], lhsT=lh, rhs=wl_sb[:, kk, :, NF:D],
                            start=(kk == 0), stop=(kk == KK - 1), perf_mode=DR,
                        )
                    nc.scalar.activation(
                        out=mN[i // 2][:, i % 2, :NF], in_=pma[:],
                        func=Ident, bias=0.0,
                    )
                    nc.vector.tensor_copy(
                        out=mN[i // 2][:, i % 2, NF:D], in_=pmb[:]
                    )

                def emit_mp(i):
                    pca = ps5.tile([P, NF], f32, tag="ps5", name=f"pca{l}_{i}")
                    pcb = ps5.tile([P, D - NF], f32, tag="ps5",
                                   name=f"pcb{l}_{i}")
                    for k in range(DCH):
                        lh = hT[k][:, i * P : (i + 1) * P]
                        nc.tensor.matmul(
                            out=pca[:], lhsT=lh, rhs=wcn_sb[k][:, :NF],
                            start=(k == 0), stop=(k == DCH - 1),
                        )
                        nc.tensor.matmul(
                            out=pcb[:], lhsT=lh, rhs=wcn_sb[k][:, NF:D],
                            start=(k == 0), stop=(k == DCH - 1),
                        )
                    nc.scalar.activation(out=mpN[i][:, :NF], in_=pca[:],
                                         func=Ident, bias=0.0)
                    nc.vector.tensor_copy(out=mpN[i][:, NF:D], in_=pcb[:])

                # m' first: it reads the f32 master h directly (no
                # dependence on the fp8 quantize tail of the previous layer);
                # contract k=0..3 first so the PE has work before the last
                # h chunks of the previous layer finish
                mp_ps = {}
                for i in range(NCH):
                    pca = ps5.tile([P, NF], f32, tag="ps5",
                                   name=f"pca{l}_{i}")
                    pcb = ps5.tile([P, D - NF], f32, tag="ps5",
                                   name=f"pcb{l}_{i}")
                    mp_ps[i] = (pca, pcb)
                    for k in range(4):
                        lh = hT[k][:, i * P : (i + 1) * P]
                        nc.tensor.matmul(
                            out=pca[:], lhsT=lh, rhs=wcn_sb[k][:, :NF],
                            start=(k == 0), stop=False,
                        )
                        nc.tensor.matmul(
                            out=pcb[:], lhsT=lh, rhs=wcn_sb[k][:, NF:D],
                            start=(k == 0), stop=False,
                        )
                    if i == 2 or i == 3:
                        for j in ((0, 1, 2) if i == 2 else (3,)):
                            pca_, pcb_ = mp_ps[j]
                            for k in range(4, DCH):
                                lh = hT[k][:, j * P : (j + 1) * P]
                                nc.tensor.matmul(
                                    out=pca_[:], lhsT=lh,
                                    rhs=wcn_sb[k][:, :NF],
                                    start=False, stop=(k == DCH - 1),
                                )
                                nc.tensor.matmul(
                                    out=pcb_[:], lhsT=lh,
                                    rhs=wcn_sb[k][:, NF:D],
                                    start=False, stop=(k == DCH - 1),
                                )
                            nc.scalar.activation(out=mpN[j][:, :NF],
                                                 in_=pca_[:], func=Ident,
                                                 bias=0.0)
                            nc.vector.tensor_copy(out=mpN[j][:, NF:D],
                                                  in_=pcb_[:])
                for i in range(NCH):
                    emit_m8(i)

                # pre-open r/z psum groups with their gh contributions so
                # the PE has work while agg8 quantizes drain
                rz_ps = {}

                def emit_gh_rz(i):
                    prs = []
                    for g in (i, DCH + i):
                        pg = ps5.tile([P, NF], f32, tag="ps5",
                                      name=f"prz{l}_{g}")
                        for kk in range(KK):
                            nc.tensor.matmul(
                                out=pg[:], lhsT=whh_sb[:, g, kk, :, :],
                                rhs=hF[kk][:],
                                start=(kk == 0), stop=False, perf_mode=DR,
                            )
                        prs.append(pg)
                    rz_ps[i] = prs

                emit_gh_rz(0)

                # agg8 = A8-contraction of m8 (fp8 DR), feature-major
                aggF = [
                    aggp.tile([P, 2, NF], fp8, tag="aggp", name=f"agg{l}_{kk}")
                    for kk in range(KK)
                ]
                for fch in range(DCH):
                    pag = ps5.tile([P, NF], f32, tag="ps5")
                    for cc in range(CC):
                        nc.tensor.matmul(
                            out=pag[:],
                            lhsT=mN[cc][:, :, fch * P : (fch + 1) * P],
                            rhs=at8_sb[:, cc, :, :],
                            start=(cc == 0), stop=(cc == CC - 1), perf_mode=DR,
                        )
                    if fch % 2 == 0:
                        nc.scalar.activation(
                            out=aggF[fch // 2][:, fch % 2, :], in_=pag[:],
                            func=Ident, bias=0.0,
                            scale=float(2.0 ** (AG - AM)),
                        )
                    else:
                        nc.vector.tensor_scalar_mul(
                            out=aggF[fch // 2][:, fch % 2, :], in0=pag[:],
                            scalar1=float(2.0 ** (AG - AM)),
                        )

                # prefetch next layer's weights while gates run
                if l + 1 < L:
                    wl_next = wlp.tile([P, KK, 2, D], fp8, tag="wlp",
                                       name=f"wl{l + 1}")
                    nc.scalar.dma_start(out=wl_next[:], in_=wl8[l + 1])
                    wcn_next = [
                        wcnp.tile([P, D], f32r, tag="wcnp",
                                  name=f"wcn{l + 1}_{k}")
                        for k in range(DCH)
                    ]
                    for k in range(DCH):
                        eng = nc.sync if k % 2 else nc.scalar
                        eng.dma_start(out=wcn_next[k][:], in_=wcn[l + 1, k])

                # GRU gates, 128 gate rows at a time (feature chunk i)
                hTn = []
                hFn = [
                    hf8.tile([P, 2, NF], fp8, tag="hf8",
                             name=f"h{l + 1}q{kk}")
                    for kk in range(KK)
                ] if l + 1 < L else []
                # stage-skewed gate pipeline: A(i) = matmuls + sigmoids,
                # B(i) = rn/tn/tanh, C(i) = h' update + fp8 shadow.  Skewing
                # keeps every engine's in-order queue dependency-ready.
                st = [dict() for _ in range(DCH)]

                def stageA(i):
                    rz = []
                    for gi_, (g, pg) in enumerate(
                        ((i, rz_ps[i][0]), (DCH + i, rz_ps[i][1]))
                    ):
                        for kk in range(KK):
                            nc.tensor.matmul(
                                out=pg[:], lhsT=wih_sb[:, g, kk, :, :],
                                rhs=aggF[kk][:],
                                start=False, stop=(kk == KK - 1), perf_mode=DR,
                            )
                        gs = gp.tile([P, NF], f32, tag="gp",
                                     name=f"g{l}_{i}_{gi_}")
                        nc.scalar.activation(
                            out=gs[:], in_=pg[:], func=Sigmoid,
                            bias=bsum_sb[:, g : g + 1],
                            scale=float(2.0 ** -SG),
                        )
                        rz.append(gs)
                    del rz_ps[i]
                    if i + 1 < DCH and i + 1 not in rz_ps:
                        emit_gh_rz(i + 1)
                    st[i]["r"], st[i]["z"] = rz

                def stageB(i):
                    s_i = st[i]
                    g = 2 * DCH + i
                    pghn = psN.tile([P, NF], f32, tag="psN",
                                    name=f"pghn{l}_{i}")
                    for kk in range(KK):
                        nc.tensor.matmul(
                            out=pghn[:], lhsT=whh_sb[:, g, kk, :, :],
                            rhs=hF[kk][:],
                            start=(kk == 0), stop=(kk == KK - 1), perf_mode=DR,
                        )
                    pgin = psN.tile([P, NF], f32, tag="psN",
                                    name=f"pgin{l}_{i}")
                    for c in range(NCH):
                        nc.tensor.matmul(
                            out=pgin[:],
                            lhsT=mpN[c][:, i * P : (i + 1) * P],
                            rhs=atf_sb[:, c, :],
                            start=(c == 0), stop=(c == NCH - 1),
                        )
                    s_i["pghn"], s_i["pgin"] = pghn, pgin
                    rn = gp.tile([P, NF], f32, tag="gp", name=f"rn{l}_{i}")
                    if has_bhhn:
                        hb = gp.tile([P, NF], f32, tag="gp",
                                     name=f"hb{l}_{i}")
                        nc.scalar.activation(
                            out=hb[:], in_=s_i["pghn"][:], func=Ident,
                            bias=bhhn_sb[:, i : i + 1], scale=1.0,
                        )
                        nc.vector.tensor_mul(out=rn[:], in0=s_i["r"][:],
                                             in1=hb[:])
                    else:
                        nc.vector.tensor_mul(out=rn[:], in0=s_i["r"][:],
                                             in1=s_i["pghn"][:])
                    tn = gp.tile([P, NF], f32, tag="gp", name=f"tn{l}_{i}")
                    nc.vector.tensor_add(out=tn[:], in0=s_i["pgin"][:],
                                         in1=rn[:])
                    nn_ = gp.tile([P, NF], f32, tag="gp", name=f"nn{l}_{i}")
                    nc.scalar.activation(
                        out=nn_[:], in_=tn[:], func=Tanh,
                        bias=bihn_sb[:, i : i + 1], scale=float(2.0 ** -SG),
                    )
                    s_i["n"] = nn_

                def stageC(i):
                    s_i = st[i]
                    nn_ = s_i["n"]
                    s_ = gp.tile([P, NF], f32, tag="gp", name=f"s{l}_{i}")
                    nc.gpsimd.tensor_sub(out=s_[:], in0=hT[i][:], in1=nn_[:])
                    sz = gp.tile([P, NF], f32, tag="gp", name=f"sz{l}_{i}")
                    nc.gpsimd.tensor_mul(out=sz[:], in0=s_i["z"][:], in1=s_[:])
                    hj = hp.tile([P, NF], f32r, tag="hp",
                                 name=f"h{l + 1}_{i}")
                    nc.vector.tensor_add(out=hj[:], in0=nn_[:], in1=sz[:])
                    hTn.append(hj)
                    if l + 1 < L:
                        if i % 2 == 0:
                            nc.scalar.activation(
                                out=hFn[i // 2][:, i % 2, :], in_=hj[:],
                                func=Ident, bias=0.0, scale=float(2 ** AH),
                            )
                        else:
                            nc.vector.tensor_scalar_mul(
                                out=hFn[i // 2][:, i % 2, :], in0=hj[:],
                                scalar1=float(2 ** AH),
                            )

                for t in range(DCH + 2):
                    if t < DCH:
                        stageA(t)
                    if 1 <= t + 0 and 0 <= t - 1 < DCH:
                        stageB(t - 1)
                    if 0 <= t - 2 < DCH:
                        stageC(t - 2)
                hT, hF = hTn, hFn
                if l + 1 < L:
                    wl_sb = wl_next
                    wcn_sb = wcn_next

            # ---- transpose back to node-major, mask, write out ----
            # j-major: transposes for feature chunk j start as soon as the
            # last layer's h'(j) lands, overlapping the pointwise drain
            poas = [ps5.tile([P, NF], f32, tag="ps5", name=f"poa{i}")
                    for i in range(NCH)]
            pobs = [psN.tile([P, D - NF], f32, tag="psN", name=f"pob{i}")
                    for i in range(NCH)]
            for j in range(DCH):
                for i in range(NCH):
                    dst = poas[i][:, j * P : (j + 1) * P] if j < 4 else \
                        pobs[i][:, (j - 4) * P : (j - 3) * P]
                    nc.tensor.transpose(
                        out=dst,
                        in_=hT[j][:, i * P : (i + 1) * P].bitcast(f32),
                        identity=identity[:],
                    )
            for i in range(NCH):
                ob = obp.tile([P, D], f32, tag="obp")
                nc.vector.tensor_scalar_mul(
                    out=ob[:, :NF], in0=poas[i][:],
                    scalar1=mask_sb[:, i : i + 1],
                )
                nc.scalar.activation(
                    out=ob[:, NF:D], in_=pobs[i][:], func=Ident, bias=0.0,
                    scale=mask_sb[:, i : i + 1],
                )
                eng = nc.sync if i % 2 else nc.scalar
                eng.dma_start(out=out[i * P : (i + 1) * P, :NF],
                              in_=ob[:, :NF])
                eng2 = nc.scalar if i % 2 else nc.sync
                eng2.dma_start(out=out[i * P : (i + 1) * P, NF:D],
                               in_=ob[:, NF:D])

    nc.compile()
    return nc


@functools.lru_cache(maxsize=4)
def _get_nc(pool_wide: bool, has_bhhn: bool) -> bass.Bass:
    return build_nc(pool_wide, has_bhhn)


def _prep_shared(inputs, pool_wide):
    """Weight tensors identical across graphs, pre-quantized / pre-laid-out
    partition-major so every DMA is contiguous per partition."""
    word = np.asarray(inputs["word_emb"], np.float32)
    tt = np.ascontiguousarray(np.asarray(inputs["type_table"], np.float32))
    fw = np.asarray(inputs["fusion_w"], np.float32)          # [F, D]
    # [p, j, c, m] = fw[c*128+p, j*128+m]
    fusion_w = np.ascontiguousarray(
        fw.reshape(FCH, P, DCH, P).transpose(1, 2, 0, 3)
    ).astype(NPBF)
    wl_w = np.asarray(inputs["ggnn_w"], np.float32)          # [L, D, D]
    wl8 = np.ascontiguousarray(
        (wl_w * 2.0 ** BWL).reshape(L, KK, 2, P, D).transpose(0, 3, 1, 2, 4)
    ).astype(NPF8)
    wih_w = np.asarray(inputs["gru_w_ih"], np.float32)       # [3D, D]
    whh_w = np.asarray(inputs["gru_w_hh"], np.float32)
    # [p, g, kk, i, m] = W.T[(2kk+i)*128+p, g*128+m] * scale
    wih8 = np.ascontiguousarray(
        (wih_w[: 2 * D].T * 2.0 ** BIH)
        .reshape(KK, 2, P, 2 * DCH, P).transpose(2, 3, 0, 1, 4)
    ).astype(NPF8)
    whh8 = np.ascontiguousarray(
        (whh_w.T * 2.0 ** BHH)
        .reshape(KK, 2, P, GCH, P).transpose(2, 3, 0, 1, 4)
    ).astype(NPF8)
    # Wcn_l = Wl @ Wih_n^T, pre-scaled by 2^SG; [l, k, p, j]
    wih_n = wih_w[2 * D :]
    wcn = np.stack([
        (wl_w[l].astype(np.float64) @ wih_n.T.astype(np.float64))
        for l in range(L)
    ]).astype(np.float32) * 2.0 ** SG
    wcn = np.ascontiguousarray(wcn.reshape(L, DCH, P, D))
    bih = np.asarray(inputs["gru_b_ih"], np.float32)
    bhh = np.asarray(inputs["gru_b_hh"], np.float32)
    smalls = np.zeros((P, 34), np.float32)
    smalls[:, 0:12] = (bih[: 2 * D] + bhh[: 2 * D]).reshape(2 * DCH, P).T
    smalls[:, 12:18] = bih[2 * D :].reshape(DCH, P).T
    smalls[:, 18:24] = (bhh[2 * D :] * 2.0 ** SG).reshape(DCH, P).T
    smalls[:, 24:30] = (
        np.asarray(inputs["fusion_b"], np.float32).reshape(DCH, P).T
    )
    shared = dict(
        type_table=tt, fusion_w=fusion_w, wl8=wl8, wcn=wcn, wih8=wih8,
        whh8=whh8, _smalls_base=smalls,
    )
    shared["word_bf"] = np.ascontiguousarray(word.astype(NPBF))
    return shared


def _graph_blockable(inputs, b):
    seg = np.asarray(inputs["token_seg_ids"][b], np.int64)
    tcol = np.arange(T) // P
    return bool(np.all((seg >= tcol * BLK) & (seg < (tcol + 1) * BLK)))


def _prep_graph(inputs, b, pool_wide):
    tok = np.asarray(inputs["node_token_ids"][b], np.int64)
    typ = np.asarray(inputs["node_types"][b], np.int32)
    seg = np.asarray(inputs["token_seg_ids"][b], np.int64)
    lens = np.asarray(inputs["node_token_lens"][b], np.float64)
    glen = int(np.asarray(inputs["graph_node_lens"][b]))
    esrc = np.asarray(inputs["edge_src"][b], np.int64)
    edst = np.asarray(inputs["edge_dst"][b], np.int64)
    ew = np.asarray(inputs["edge_weight"][b], np.float32)

    # token idxs for dma_gather: GS splits of GT idxs, each wrapped into
    # 16 partitions and replicated to 128 partitions
    tok16 = tok.astype(np.int16)
    cols = []
    for s in range(GS):
        w16 = tok16[s * GT : (s + 1) * GT].reshape(GT // 16, 16).T
        cols.append(np.tile(w16, (8, 1)))
    tok_idx = np.ascontiguousarray(np.concatenate(cols, axis=1))

    typ_oh = np.zeros((TYPES, N), np.float32)
    typ_oh[typ, np.arange(N)] = 1.0

    # dense adjacency A[src, dst]: f32r copy + DoubleRow-paired fp8 copy
    at = np.zeros((N, N), np.float32)
    np.add.at(at, (esrc, edst), ew)
    at_f = np.ascontiguousarray(
        at.reshape(NCH, P, N).transpose(1, 0, 2)
    ).astype(NPBF)
    at_8 = np.ascontiguousarray(
        at.reshape(CC, 2, P, N).transpose(2, 0, 1, 3)
    ).astype(NPF8)

    winv = np.zeros(N, np.float64)
    nzmask = lens != 0
    winv[nzmask] = 1.0 / lens[nzmask]
    t_ = np.arange(T)
    c_ = t_ // P
    if pool_wide:
        pm = np.zeros((TCH, P, N), np.float32)
        pm[c_, t_ % P, seg] = winv[seg]
        pool_arr = np.ascontiguousarray(pm.transpose(1, 0, 2)).astype(NPBF)
        pool_key = "poolw"
    else:
        pm = np.zeros((TCH, P, BLK), np.float32)
        pm[c_, t_ % P, seg - c_ * BLK] = winv[seg]
        pool_arr = np.ascontiguousarray(pm.transpose(1, 0, 2)).astype(NPBF)
        pool_key = "poolm"

    keep = min(glen, MAX_NODE_LEN)
    mask = np.ascontiguousarray(
        (np.arange(N) < keep).astype(np.float32).reshape(NCH, P).T
    )
    return {
        "tok_idx": tok_idx, "typ_oh": typ_oh, "at_f": at_f, "at_8": at_8,
        pool_key: pool_arr, "_mask": mask,
    }


def kernel(**inputs) -> np.ndarray:
    pool_wide = not all(_graph_blockable(inputs, b) for b in range(B))
    has_bhhn = bool(
        np.any(np.asarray(inputs["gru_b_hh"], np.float32)[2 * D :] != 0.0)
    )
    shared = _prep_shared(inputs, pool_wide)
    smalls_base = shared.pop("_smalls_base")
    per_graph = [_prep_graph(inputs, b, pool_wide) for b in range(B)]
    in_maps = []
    for b in range(B):
        g = dict(per_graph[b])
        sm = smalls_base.copy()
        sm[:, 30:34] = g.pop("_mask")
        g["smalls"] = sm
        in_maps.append({**shared, **g})
    nc = _get_nc(pool_wide, has_bhhn)
    res = bass_utils.run_bass_kernel_spmd(nc, in_maps, core_ids=list(range(B)))
    global _last_exec_ns
    _last_exec_ns = res.exec_time_ns
    out = np.stack([r["out"] for r in res.results]).astype(np.float32)
    return out


_last_exec_ns = None
_last_nc = None
